# revision 34
# baseline (speedup 1.0000x reference)
"""Causal self-attention v7 (B=4, T=2048, C=1024, H=16) on 8 trn2 NeuronCores.

Sharding: core = (batch b, head-group hg), b = core//2, hg = core%2 (Megatron
column-parallel qkv / row-parallel proj); host sums the two partial outputs.

Changes vs the 468us v2 baseline (measured ~283us, la=5/ptbufs=8):
  - qb-major schedule: proj tasks (a=QK, b=V, d=out-proj) woven INTO the
    attention group stream so the PE never idles while ACT (exp) works;
    d(qb-1) runs during C(qb).
  - normalization: DVE copy of the PSUM rowsum row (recip straight off PSUM
    is broken on HW - sim diverges), DVE reciprocal, GPSIMD
    partition_broadcast (idle engine) for the [64,512] broadcast, single DVE
    mul per head.  No more PE broadcast matmuls.
  - S/pt tiles are [128, 2, 512] so straddle groups get ONE exp over a
    strided AP instead of two (saves 352 ACT cycles per instruction).
  - proj PSUM tiles are one bank each ([128,512]) in their own pool, so an
    in-flight a/b/d task no longer blocks the S-tile pipeline.
  - PSUM budget: s 2x2 banks + y 2x1 + proj 2x1 = 8 banks.
  - yt stored per-pair (finer dep granularity for the tail d-tasks); v ones
    columns memset only (was a 7us whole-tile memset blocking DVE each rep).
  - batched DMAs (rearranged whole-tensor transfers, one out-DMA per
    t-tile): each dma_start costs ~600ns of HWDGE issue time.
  - fp8a (DoubleRow QK-projection) exists but is OFF: e4m3 q/k noise puts
    max-rel err at 3.5e-2 > 2e-2 tolerance.
  - lookahead la=5 with ptbufs=8 (exp/mask run ~5 groups ahead of the AV
    matmuls): rep3-trace sweep la=3/4/5/6 -> 844.8/833.8/827.6/832.6us.
"""

import functools

import numpy as np

B, T, C, H = 4, 2048, 1024, 16
HD = C // H  # 64
N_CORES = 8
HG = 2  # head groups
NH = H // HG  # heads per core = 8
NP = NH // 2  # head pairs per core = 4
TT = T // 128  # 16 t-tiles
TB = T // 512  # 4 t-blocks
CK = C // 128  # 8 c-chunks


def _build(rep=1, la=5, sbufs=2, ybufs=2, pbufs=2, ptbufs=8, ablate="full",
           norm="sbuf", fp8a=False, xbufs=1, vbufs=1):
    import concourse.bass as bass
    import concourse.mybir as mybir
    import concourse.tile as tile
    from concourse import bacc

    f32 = mybir.dt.float32
    bf16 = mybir.dt.bfloat16
    f8 = mybir.dt.float8e4

    nc = bacc.Bacc("TRN2", target_bir_lowering=False, debug=False)

    xt_d = nc.dram_tensor("xt", [C, T], bf16, kind="ExternalInput")
    if fp8a:
        # fp8 copies for the DoubleRow QK projection; wq8/wk8 are already
        # interleaved host-side as [ki=128, kpair=4, j=2, m=512] and carry a
        # x32 pre-scale (compensated in the exp scale) to clear the e4m3
        # subnormal range.
        xt8_d = nc.dram_tensor("xt8", [C, T], f8, kind="ExternalInput")
        wq8_d = nc.dram_tensor("wq8", [128, 4096], f8, kind="ExternalInput")
        wk8_d = nc.dram_tensor("wk8", [128, 4096], f8, kind="ExternalInput")
    else:
        wq_d = nc.dram_tensor("wq", [C, 512], bf16, kind="ExternalInput")
        wk_d = nc.dram_tensor("wk", [C, 512], bf16, kind="ExternalInput")
    wv_d = nc.dram_tensor("wv", [C, 512], bf16, kind="ExternalInput")
    wp_d = nc.dram_tensor("wp", [512, C], bf16, kind="ExternalInput")
    tri_d = nc.dram_tensor("tri", [128, 128], bf16, kind="ExternalInput")
    out_d = nc.dram_tensor("out", [T, C], f32, kind="ExternalOutput")

    exp_scale = 0.125 / 1024.0 if fp8a else 0.125

    do_attn = ablate in ("full", "noav")
    do_av = ablate in ("full",)
    do_d = ablate in ("full", "noattn")

    with tile.TileContext(nc) as tc:
        with tc.tile_pool(name="persist", bufs=1) as persist:
            qt_sb = persist.tile([128, NP, T], bf16, tag="qt")
            kt_sb = persist.tile([128, NP, T], bf16, tag="kt")

            def body():
                # strictly nested (LIFO) pool lifetimes
                vp_cm = tc.tile_pool(name="vp", bufs=vbufs)
                xtp_cm = tc.tile_pool(name="xtp", bufs=xbufs)
                wqk_cm = tc.tile_pool(name="wqk", bufs=1)
                vp = vp_cm.__enter__()
                xtp = xtp_cm.__enter__()
                wqk = wqk_cm.__enter__()

                v_sb = vp.tile([128, TT, NH, HD + 1], bf16, tag="v")
                # ones columns of V' only (V-proj copies fill 0:HD)
                nc.vector.memset(v_sb[:, :, :, HD:HD + 1], 1.0)

                if fp8a:
                    wq8_sb = wqk.tile([128, 4, 2, 512], f8, tag="wq8")
                    wk8_sb = wqk.tile([128, 4, 2, 512], f8, tag="wk8")
                    xt8_sb = xtp.tile([128, CK, T], f8, tag="xt8")
                else:
                    wq_sb = wqk.tile([128, CK, 512], bf16, tag="wq")
                    wk_sb = wqk.tile([128, CK, 512], bf16, tag="wk")
                wv_sb = wqk.tile([128, CK, 512], bf16, tag="wv")
                wp_sb = wqk.tile([128, NP, C], bf16, tag="wp")
                tri_sb = wqk.tile([128, 128], bf16, tag="tri")
                xt_sb = xtp.tile([128, CK, T], bf16, tag="xt")
                # exp-table preload on the idle ACT engine during the DMA head
                warm = wqk.tile([128, 32], bf16, tag="warm")
                nc.vector.memset(warm[:], 1.0)
                nc.scalar.activation(
                    warm[0:1, 16:32], warm[0:1, 0:16],
                    mybir.ActivationFunctionType.Exp, scale=exp_scale,
                )
                # chunked DMAs in consumption order; tri first (first-unit
                # masks); QK operands for tb=0 first so phase A starts early.
                nc.sync.dma_start(tri_sb[:], tri_d[:, :])
                if fp8a:
                    nc.sync.dma_start(wq8_sb[:], wq8_d.rearrange(
                        "p (a b n) -> p a b n", a=4, b=2))
                    nc.sync.dma_start(wk8_sb[:], wk8_d.rearrange(
                        "p (a b n) -> p a b n", a=4, b=2))
                    for tb in range(TB):
                        tsl = slice(tb * 512, (tb + 1) * 512)
                        for k in range(CK):
                            ksl = slice(k * 128, (k + 1) * 128)
                            nc.sync.dma_start(
                                xt8_sb[:, k, tsl], xt8_d[ksl, tsl])
                    for k in range(CK):
                        ksl = slice(k * 128, (k + 1) * 128)
                        nc.sync.dma_start(xt_sb[:, k, 0:512], xt_d[ksl, 0:512])
                        nc.sync.dma_start(wv_sb[:, k, :], wv_d[ksl, :])
                else:
                    nc.sync.dma_start(
                        wq_sb[:], wq_d.rearrange("(a p) n -> p a n", p=128))
                    nc.sync.dma_start(
                        wk_sb[:], wk_d.rearrange("(a p) n -> p a n", p=128))
                    nc.sync.dma_start(
                        xt_sb[:, :, 0:512],
                        xt_d[:, 0:512].rearrange("(a p) t -> p a t", p=128))
                    nc.sync.dma_start(
                        wv_sb[:], wv_d.rearrange("(a p) n -> p a n", p=128))
                for tb in range(1, TB):
                    tsl = slice(tb * 512, (tb + 1) * 512)
                    nc.sync.dma_start(
                        xt_sb[:, :, tsl],
                        xt_d[:, tsl].rearrange("(a p) t -> p a t", p=128))
                nc.sync.dma_start(wp_sb[:], wp_d.rearrange("(a p) n -> p a n", p=128))

                with (
                    tc.tile_pool(name="persist2", bufs=1) as persist2,
                    tc.tile_pool(name="ptp", bufs=ptbufs) as ptp,
                    tc.tile_pool(name="recp", bufs=2) as recp,
                    tc.tile_pool(name="bcp", bufs=2) as bcp,
                    tc.tile_pool(name="outp", bufs=4) as outp,
                ):
                    yt_sb = [
                        persist2.tile([128, T], bf16, tag=f"yt{p}",
                                      name=f"yt{p}")
                        for p in range(NP)
                    ]
                    if ablate in ("noattn",):
                        for p in range(NP):
                            nc.vector.memset(yt_sb[p][:], 0.001)
                    with (
                        tc.tile_pool(name="pss", bufs=sbufs, space="PSUM") as pss,
                        tc.tile_pool(name="psy", bufs=ybufs, space="PSUM") as psy,
                        tc.tile_pool(name="psp", bufs=pbufs, space="PSUM") as psp,
                    ):
                        # ---- task list: qb-major, proj tasks woven in
                        tasks = []
                        if do_attn:
                            for pp in range(NP):
                                tasks.append(("a", pp, 0, 0, 0))
                            for tt in range(4):
                                tasks.append(("b", tt, 0, 0, 0))
                            for qb in range(TB):
                                inter = []
                                if qb < TB - 1:
                                    for pp in range(NP):
                                        inter.append(("a", pp, qb + 1, 0, 0))
                                    for tt in range(4 * qb + 4, 4 * qb + 8):
                                        inter.append(("b", tt, 0, 0, 0))
                                if do_d and qb > 0:
                                    for tt in range(4 * (qb - 1), 4 * qb):
                                        inter.append(("d", tt, 0, 0, 0))
                                ng = 4 * (qb + 1)
                                glist = [
                                    ("g", qb, p, g, ng)
                                    for p in range(NP)
                                    for g in range(ng)
                                ]
                                if inter:
                                    step = max(1, len(glist) // len(inter))
                                    woven, ii = [], 0
                                    for j, t in enumerate(glist):
                                        woven.append(t)
                                        if j % step == step - 1 and ii < len(inter):
                                            woven.append(inter[ii])
                                            ii += 1
                                    woven += inter[ii:]
                                    glist = woven
                                tasks += glist
                            if do_d:
                                for tt in range(TT - 4, TT):
                                    tasks.append(("d", tt, 0, 0, 0))
                        elif do_d:
                            for tt in range(TT):
                                tasks.append(("d", tt, 0, 0, 0))

                        pt_store = {}
                        s_store = {}
                        y_store = {}

                        def emit_front(idx):
                            kind, qb, p, g, ng = tasks[idx]
                            if kind != "g":
                                return
                            r = g - 4 * qb  # >=0: diagonal-straddling chunk
                            lo = 128 * r if r > 0 else 0
                            ksl = slice(g * 128, (g + 1) * 128)
                            s = pss.tile([128, 2, 512], f32, tag="s",
                                         name=f"s_{idx}")
                            for hf in range(2):
                                nc.tensor.matmul(
                                    s[:, hf, lo:512],
                                    kt_sb[64 * hf: 64 * (hf + 1), p, ksl],
                                    qt_sb[64 * hf: 64 * (hf + 1), p,
                                          qb * 512 + lo: (qb + 1) * 512],
                                    start=True, stop=True,
                                )
                            pt = ptp.tile([128, 2, 512], bf16, tag="pt",
                                          name=f"pt_{idx}")
                            # one exp per group (strided AP when straddling)
                            nc.scalar.activation(
                                pt[:, :, lo:512], s[:, :, lo:512],
                                mybir.ActivationFunctionType.Exp,
                                scale=exp_scale,
                            )
                            if r >= 0:
                                # triangle mask on the diagonal 128-wide strip
                                for hf in range(2):
                                    nc.vector.tensor_mul(
                                        pt[:, hf, lo:lo + 128],
                                        pt[:, hf, lo:lo + 128], tri_sb[:])
                            pt_store[idx] = pt

                        def emit_back(idx):
                            kind, qb, p, g, ng = tasks[idx]
                            if kind == "a":
                                pp, tb = qb, p
                                tsl = slice(tb * 512, (tb + 1) * 512)
                                psl = slice(pp * 128, (pp + 1) * 128)
                                psq = psp.tile([128, 512], f32, tag="o",
                                               name=f"psq{pp}_{tb}")
                                if fp8a:
                                    for kp in range(4):
                                        nc.tensor.matmul(
                                            psq[:], wq8_sb[:, kp, :, psl],
                                            xt8_sb[:, 2 * kp:2 * kp + 2, tsl],
                                            start=(kp == 0), stop=(kp == 3),
                                            perf_mode=(
                                                mybir.MatmulPerfMode.DoubleRow),
                                        )
                                else:
                                    for k in range(CK):
                                        nc.tensor.matmul(
                                            psq[:], wq_sb[:, k, psl],
                                            xt_sb[:, k, tsl],
                                            start=(k == 0), stop=(k == CK - 1),
                                        )
                                nc.vector.tensor_copy(qt_sb[:, pp, tsl], psq[:])
                                psk = psp.tile([128, 512], f32, tag="o",
                                               name=f"psk{pp}_{tb}")
                                if fp8a:
                                    for kp in range(4):
                                        nc.tensor.matmul(
                                            psk[:], wk8_sb[:, kp, :, psl],
                                            xt8_sb[:, 2 * kp:2 * kp + 2, tsl],
                                            start=(kp == 0), stop=(kp == 3),
                                            perf_mode=(
                                                mybir.MatmulPerfMode.DoubleRow),
                                        )
                                else:
                                    for k in range(CK):
                                        nc.tensor.matmul(
                                            psk[:], wk_sb[:, k, psl],
                                            xt_sb[:, k, tsl],
                                            start=(k == 0), stop=(k == CK - 1),
                                        )
                                nc.vector.tensor_copy(kt_sb[:, pp, tsl], psk[:])
                                return
                            if kind == "b":
                                tt = qb
                                psv = psp.tile([128, 512], f32, tag="o",
                                               name=f"psv{tt}")
                                for k in range(CK):
                                    nc.tensor.matmul(
                                        psv[:],
                                        xt_sb[:, k, tt * 128: (tt + 1) * 128],
                                        wv_sb[:, k, :],
                                        start=(k == 0), stop=(k == CK - 1),
                                    )
                                nc.vector.tensor_copy(
                                    v_sb[:, tt, :, 0:HD],
                                    psv[:].rearrange("p (h e) -> p h e", e=HD),
                                )
                                return
                            if kind == "d":
                                tt = qb
                                tsl = slice(tt * 128, (tt + 1) * 128)
                                ot = outp.tile([128, 1024], f32, tag="ot")
                                for nb in range(2):
                                    po = psp.tile([128, 512], f32, tag="o",
                                                  name=f"po{tt}_{nb}")
                                    for pp2 in range(NP):
                                        nc.tensor.matmul(
                                            po[:],
                                            yt_sb[pp2][:, tsl],
                                            wp_sb[:, pp2,
                                                  nb * 512: (nb + 1) * 512],
                                            start=(pp2 == 0),
                                            stop=(pp2 == NP - 1),
                                        )
                                    nc.vector.tensor_copy(
                                        ot[:, nb * 512: (nb + 1) * 512], po[:])
                                nc.sync.dma_start(out_d[tsl, :], ot[:])
                                return
                            # kind == "g"
                            r = g - 4 * qb
                            lo = 128 * r if r > 0 else 0
                            pt = pt_store.pop(idx)
                            if (qb, p) not in y_store:
                                ya = psy.tile([65, 512], f32, tag="y",
                                              name=f"ya_{qb}_{p}")
                                yb = psy.tile([65, 512], f32, tag="y",
                                              name=f"yb_{qb}_{p}")
                                y_store[(qb, p)] = (ya, yb)
                            ya, yb = y_store[(qb, p)]
                            if do_av:
                                for hf, yy in ((0, ya), (1, yb)):
                                    nc.tensor.matmul(
                                        yy[:, lo:512],
                                        v_sb[:, g, 2 * p + hf, :],
                                        pt[:, hf, lo:512],
                                        start=(g == 0), stop=(g == ng - 1),
                                        skip_group_check=True,
                                    )
                            if g != ng - 1:
                                return
                            if not do_av:
                                nc.vector.memset(ya[:], 1.0)
                                nc.vector.memset(yb[:], 1.0)
                            # normalize: yt = y[0:64] * (1 / rowsum)
                            qsl = slice(qb * 512, (qb + 1) * 512)
                            for hi, yy in ((0, ya), (1, yb)):
                                rec = recp.tile([1, 512], f32, tag="rec",
                                                name=f"rec_{qb}_{p}_{hi}")
                                if norm == "gps":
                                    nc.vector.reciprocal_approx_fast(
                                        rec[0:1, :], yy[64:65, :])
                                else:  # "sbuf": stage rowsum in SBUF first
                                    rs = recp.tile([1, 512], f32, tag="rs",
                                                   name=f"rs_{qb}_{p}_{hi}")
                                    nc.vector.tensor_copy(
                                        rs[0:1, :], yy[64:65, :])
                                    nc.vector.reciprocal_approx_fast(
                                        rec[0:1, :], rs[0:1, :])
                                bc = bcp.tile([64, 512], f32, tag="bc",
                                              name=f"bc_{qb}_{p}_{hi}")
                                nc.gpsimd.partition_broadcast(
                                    bc[:], rec[0:1, :], channels=64)
                                nc.vector.tensor_mul(
                                    yt_sb[p][hi * 64: (hi + 1) * 64, qsl],
                                    yy[0:64, :], bc[:],
                                )
                            del y_store[(qb, p)]

                        n = len(tasks)
                        for j in range(min(la, n)):
                            emit_front(j)
                        for i in range(n):
                            if i + la < n:
                                emit_front(i + la)
                            emit_back(i)

                wqk_cm.__exit__(None, None, None)
                xtp_cm.__exit__(None, None, None)
                vp_cm.__exit__(None, None, None)

            if rep == 1:
                body()
            else:
                with tc.For_i(0, rep, 1):
                    body()

    nc.compile()
    return nc


@functools.lru_cache(maxsize=None)
def _get_nc(rep=1, la=5, sbufs=2, ybufs=2, pbufs=2, ptbufs=8, ablate="full",
            norm="sbuf", fp8a=False, xbufs=1, vbufs=1):
    return _build(rep, la, sbufs, ybufs, pbufs, ptbufs, ablate, norm, fp8a,
                  xbufs, vbufs)


FP8A = False  # must match the _build/_get_runner default


def make_in_maps(x, w_qkv, w_proj):
    import ml_dtypes
    bf16 = ml_dtypes.bfloat16
    j = np.arange(128)[None, :]
    i = np.arange(128)[:, None]
    tri = (j >= i).astype(bf16)

    in_maps = []
    for core in range(N_CORES):
        b, hg = divmod(core, HG)
        sl = slice(hg * 512, (hg + 1) * 512)
        xtb = np.ascontiguousarray(x[b].T)
        wqt = np.ascontiguousarray(w_qkv[sl].T)
        wkt = np.ascontiguousarray(w_qkv[C:2 * C][sl].T)
        m = {
            "xt": xtb.astype(bf16),
            "wq": wqt.astype(bf16),
            "wk": wkt.astype(bf16),
            "wv": np.ascontiguousarray(w_qkv[2 * C:3 * C][sl].T).astype(bf16),
            "wp": np.ascontiguousarray(w_proj[:, sl].T).astype(bf16),
            "tri": tri,
        }
        if FP8A:
            f8 = ml_dtypes.float8_e4m3

            def dr8(wt):
                # [C,512] -> DoubleRow-interleaved [128, kp*j*m] fp8, x32
                return np.ascontiguousarray(
                    (wt * 32.0).reshape(4, 2, 128, 512).transpose(2, 0, 1, 3)
                    .reshape(128, 4096)).astype(f8)

            m["xt8"] = xtb.astype(f8)
            m["wq8"] = dr8(wqt)
            m["wk8"] = dr8(wkt)
        in_maps.append(m)
    return in_maps


def combine(results):
    out = np.empty((B, T, C), dtype=np.float32)
    for b in range(B):
        out[b] = results[2 * b]["out"] + results[2 * b + 1]["out"]
    return out


# ---------------------------------------------------------------------------
# PJRT runner (device-resident inputs, reusable jitted executable)
# ---------------------------------------------------------------------------

class _Runner:
    def __init__(self, nc, n_cores=N_CORES):
        import jax
        import concourse.mybir as mybir
        from concourse import bass2jax
        from jax.sharding import Mesh, PartitionSpec, NamedSharding
        from jax.experimental.shard_map import shard_map

        self.jax = jax
        bass2jax.install_neuronx_cc_hook()
        partition_name = (
            nc.partition_id_tensor.name if nc.partition_id_tensor else None
        )
        in_names, out_names, out_avals, zero_outs = [], [], [], []
        for alloc in nc.m.functions[0].allocations:
            if not isinstance(alloc, mybir.MemoryLocationSet):
                continue
            name = alloc.memorylocations[0].name
            if alloc.kind == "ExternalInput":
                if name != partition_name:
                    in_names.append(name)
            elif alloc.kind == "ExternalOutput":
                out_names.append(name)
                shape = tuple(alloc.tensor_shape)
                dtype = mybir.dt.np(alloc.dtype)
                out_avals.append(jax.core.ShapedArray(shape, dtype))
                zero_outs.append(np.zeros(shape, dtype))
        self.in_names, self.out_names = in_names, out_names
        self.out_avals, self.zero_outs = out_avals, zero_outs
        self.n_cores = n_cores
        all_names = in_names + out_names
        if partition_name is not None:
            all_names = all_names + [partition_name]

        def _bdy(*args):
            operands = list(args)
            if partition_name is not None:
                operands.append(bass2jax.partition_id_tensor())
            outs = bass2jax._bass_exec_p.bind(
                *operands,
                out_avals=tuple(out_avals),
                in_names=tuple(all_names),
                out_names=tuple(out_names),
                lowering_input_output_aliases=(),
                sim_require_finite=True,
                sim_require_nnan=True,
                nc=nc,
            )
            return tuple(outs)

        devices = jax.devices()[:n_cores]
        mesh = Mesh(np.asarray(devices), ("core",))
        n_args = len(in_names) + len(out_names)
        self.fn = jax.jit(
            shard_map(
                _bdy, mesh=mesh,
                in_specs=(PartitionSpec("core"),) * n_args,
                out_specs=(PartitionSpec("core"),) * len(out_names),
                check_rep=False,
            ),
            keep_unused=True,
        )
        self.sharding = NamedSharding(mesh, PartitionSpec("core"))

    def put_inputs(self, in_maps):
        concat = [
            np.concatenate([np.asarray(m[name]) for m in in_maps], axis=0)
            for name in self.in_names
        ]
        concat += [
            np.zeros((self.n_cores * z.shape[0], *z.shape[1:]), z.dtype)
            for z in self.zero_outs
        ]
        self.args = [self.jax.device_put(a, self.sharding) for a in concat]
        self.jax.block_until_ready(self.args)

    def run(self):
        outs = self.fn(*self.args)
        self.jax.block_until_ready(outs)
        return [
            {
                name: np.asarray(outs[i]).reshape(
                    self.n_cores, *self.out_avals[i].shape)[c]
                for i, name in enumerate(self.out_names)
            }
            for c in range(self.n_cores)
        ]

    def time_ns(self, iters=20, warmup=2):
        import time
        for _ in range(warmup):
            self.jax.block_until_ready(self.fn(*self.args))
        t0 = time.perf_counter()
        outs = None
        for _ in range(iters):
            outs = self.fn(*self.args)
        self.jax.block_until_ready(outs)
        t1 = time.perf_counter()
        return (t1 - t0) / iters * 1e9



@functools.lru_cache(maxsize=None)
def _get_runner(rep=1, la=5, sbufs=2, ybufs=2, pbufs=2, ptbufs=8, ablate="full",
                norm="sbuf", fp8a=False, xbufs=1, vbufs=1):
    return _Runner(_get_nc(rep, la, sbufs, ybufs, pbufs, ptbufs, ablate, norm,
                           fp8a, xbufs, vbufs))


def kernel(x, w_qkv, w_proj):
    x = np.asarray(x, dtype=np.float32)
    w_qkv = np.asarray(w_qkv, dtype=np.float32)
    w_proj = np.asarray(w_proj, dtype=np.float32)
    runner = _get_runner()
    runner.put_inputs(make_in_maps(x, w_qkv, w_proj))
    return combine(runner.run())


# revision 36
# speedup vs baseline: 1.0039x; 1.0039x over previous
"""Causal self-attention v7 (B=4, T=2048, C=1024, H=16) on 8 trn2 NeuronCores.

Sharding: core = (batch b, head-group hg), b = core//2, hg = core%2 (Megatron
column-parallel qkv / row-parallel proj); host sums the two partial outputs.

Changes vs the 468us v2 baseline (measured ~283us, la=5/ptbufs=8):
  - qb-major schedule: proj tasks (a=QK, b=V, d=out-proj) woven INTO the
    attention group stream so the PE never idles while ACT (exp) works;
    d(qb-1) runs during C(qb).
  - normalization: DVE copy of the PSUM rowsum row (recip straight off PSUM
    is broken on HW - sim diverges), DVE reciprocal, GPSIMD
    partition_broadcast (idle engine) for the [64,512] broadcast, single DVE
    mul per head.  No more PE broadcast matmuls.
  - S/pt tiles are [128, 2, 512] so straddle groups get ONE exp over a
    strided AP instead of two (saves 352 ACT cycles per instruction).
  - proj PSUM tiles are one bank each ([128,512]) in their own pool, so an
    in-flight a/b/d task no longer blocks the S-tile pipeline.
  - PSUM budget: s 2x2 banks + y 2x1 + proj 2x1 = 8 banks.
  - yt stored per-pair (finer dep granularity for the tail d-tasks); v ones
    columns memset only (was a 7us whole-tile memset blocking DVE each rep).
  - batched DMAs (rearranged whole-tensor transfers, one out-DMA per
    t-tile): each dma_start costs ~600ns of HWDGE issue time.
  - fp8a (DoubleRow QK-projection) exists but is OFF: e4m3 q/k noise puts
    max-rel err at 3.5e-2 > 2e-2 tolerance.
  - lookahead la=5 with ptbufs=8 (exp/mask run ~5 groups ahead of the AV
    matmuls): rep3-trace sweep la=3/4/5/6 -> 844.8/833.8/827.6/832.6us.
"""

import functools

import numpy as np

B, T, C, H = 4, 2048, 1024, 16
HD = C // H  # 64
N_CORES = 8
HG = 2  # head groups
NH = H // HG  # heads per core = 8
NP = NH // 2  # head pairs per core = 4
TT = T // 128  # 16 t-tiles
TB = T // 512  # 4 t-blocks
CK = C // 128  # 8 c-chunks


def _build(rep=1, la=5, sbufs=2, ybufs=2, pbufs=2, ptbufs=8, ablate="full",
           norm="sbuf", fp8a=False, xbufs=1, vbufs=1):
    import concourse.bass as bass
    import concourse.mybir as mybir
    import concourse.tile as tile
    from concourse import bacc

    f32 = mybir.dt.float32
    bf16 = mybir.dt.bfloat16
    f8 = mybir.dt.float8e4

    nc = bacc.Bacc("TRN2", target_bir_lowering=False, debug=False)

    xt_d = nc.dram_tensor("xt", [C, T], bf16, kind="ExternalInput")
    if fp8a:
        # fp8 copies for the DoubleRow QK projection; wq8/wk8 are already
        # interleaved host-side as [ki=128, kpair=4, j=2, m=512] and carry a
        # x32 pre-scale (compensated in the exp scale) to clear the e4m3
        # subnormal range.
        xt8_d = nc.dram_tensor("xt8", [C, T], f8, kind="ExternalInput")
        wq8_d = nc.dram_tensor("wq8", [128, 4096], f8, kind="ExternalInput")
        wk8_d = nc.dram_tensor("wk8", [128, 4096], f8, kind="ExternalInput")
    else:
        wq_d = nc.dram_tensor("wq", [C, 512], bf16, kind="ExternalInput")
        wk_d = nc.dram_tensor("wk", [C, 512], bf16, kind="ExternalInput")
    wv_d = nc.dram_tensor("wv", [C, 512], bf16, kind="ExternalInput")
    wp_d = nc.dram_tensor("wp", [512, C], bf16, kind="ExternalInput")
    tri_d = nc.dram_tensor("tri", [128, 128], bf16, kind="ExternalInput")
    out_d = nc.dram_tensor("out", [T, C], f32, kind="ExternalOutput")

    exp_scale = 0.125 / 1024.0 if fp8a else 0.125

    do_attn = ablate in ("full", "noav")
    do_av = ablate in ("full",)
    do_d = ablate in ("full", "noattn")

    with tile.TileContext(nc) as tc:
        with tc.tile_pool(name="persist", bufs=1) as persist:
            qt_sb = persist.tile([128, NP, T], bf16, tag="qt")
            kt_sb = persist.tile([128, NP, T], bf16, tag="kt")

            def body():
                # strictly nested (LIFO) pool lifetimes
                vp_cm = tc.tile_pool(name="vp", bufs=vbufs)
                xtp_cm = tc.tile_pool(name="xtp", bufs=xbufs)
                wqk_cm = tc.tile_pool(name="wqk", bufs=1)
                vp = vp_cm.__enter__()
                xtp = xtp_cm.__enter__()
                wqk = wqk_cm.__enter__()

                v_sb = vp.tile([128, TT, NH, HD + 1], bf16, tag="v")
                # ones columns of V' only (V-proj copies fill 0:HD)
                nc.vector.memset(v_sb[:, :, :, HD:HD + 1], 1.0)

                if fp8a:
                    wq8_sb = wqk.tile([128, 4, 2, 512], f8, tag="wq8")
                    wk8_sb = wqk.tile([128, 4, 2, 512], f8, tag="wk8")
                    xt8_sb = xtp.tile([128, CK, T], f8, tag="xt8")
                else:
                    wq_sb = wqk.tile([128, CK, 512], bf16, tag="wq")
                    wk_sb = wqk.tile([128, CK, 512], bf16, tag="wk")
                wv_sb = wqk.tile([128, CK, 512], bf16, tag="wv")
                wp_sb = wqk.tile([128, NP, C], bf16, tag="wp")
                tri_sb = wqk.tile([128, 128], bf16, tag="tri")
                xt_sb = xtp.tile([128, CK, T], bf16, tag="xt")
                # exp-table preload on the idle ACT engine during the DMA head
                warm = wqk.tile([128, 32], bf16, tag="warm")
                nc.vector.memset(warm[:], 1.0)
                nc.scalar.activation(
                    warm[0:1, 16:32], warm[0:1, 0:16],
                    mybir.ActivationFunctionType.Exp, scale=exp_scale,
                )
                # chunked DMAs in consumption order; tri first (first-unit
                # masks); QK operands for tb=0 first so phase A starts early.
                nc.sync.dma_start(tri_sb[:], tri_d[:, :])
                if fp8a:
                    nc.sync.dma_start(wq8_sb[:], wq8_d.rearrange(
                        "p (a b n) -> p a b n", a=4, b=2))
                    nc.sync.dma_start(wk8_sb[:], wk8_d.rearrange(
                        "p (a b n) -> p a b n", a=4, b=2))
                    for tb in range(TB):
                        tsl = slice(tb * 512, (tb + 1) * 512)
                        for k in range(CK):
                            ksl = slice(k * 128, (k + 1) * 128)
                            nc.sync.dma_start(
                                xt8_sb[:, k, tsl], xt8_d[ksl, tsl])
                    for k in range(CK):
                        ksl = slice(k * 128, (k + 1) * 128)
                        nc.sync.dma_start(xt_sb[:, k, 0:512], xt_d[ksl, 0:512])
                        nc.sync.dma_start(wv_sb[:, k, :], wv_d[ksl, :])
                else:
                    nc.sync.dma_start(
                        wq_sb[:], wq_d.rearrange("(a p) n -> p a n", p=128))
                    nc.sync.dma_start(
                        wk_sb[:], wk_d.rearrange("(a p) n -> p a n", p=128))
                    nc.sync.dma_start(
                        xt_sb[:, :, 0:512],
                        xt_d[:, 0:512].rearrange("(a p) t -> p a t", p=128))
                    nc.sync.dma_start(
                        wv_sb[:], wv_d.rearrange("(a p) n -> p a n", p=128))
                for tb in range(1, TB):
                    tsl = slice(tb * 512, (tb + 1) * 512)
                    nc.sync.dma_start(
                        xt_sb[:, :, tsl],
                        xt_d[:, tsl].rearrange("(a p) t -> p a t", p=128))
                nc.sync.dma_start(wp_sb[:], wp_d.rearrange("(a p) n -> p a n", p=128))

                with (
                    tc.tile_pool(name="persist2", bufs=1) as persist2,
                    tc.tile_pool(name="ptp", bufs=ptbufs) as ptp,
                    tc.tile_pool(name="recp", bufs=2) as recp,
                    tc.tile_pool(name="bcp", bufs=2) as bcp,
                    tc.tile_pool(name="outp", bufs=4) as outp,
                ):
                    yt_sb = [
                        persist2.tile([128, T], bf16, tag=f"yt{p}",
                                      name=f"yt{p}")
                        for p in range(NP)
                    ]
                    if ablate in ("noattn",):
                        for p in range(NP):
                            nc.vector.memset(yt_sb[p][:], 0.001)
                    with (
                        tc.tile_pool(name="pss", bufs=sbufs, space="PSUM") as pss,
                        tc.tile_pool(name="psy", bufs=ybufs, space="PSUM") as psy,
                        tc.tile_pool(name="psp", bufs=pbufs, space="PSUM") as psp,
                    ):
                        # ---- task list: qb-major, proj tasks woven in
                        tasks = []
                        if do_attn:
                            for pp in range(NP):
                                tasks.append(("a", pp, 0, 0, 0))
                            for tt in range(4):
                                tasks.append(("b", tt, 0, 0, 0))
                            for qb in range(TB):
                                inter = []
                                if qb < TB - 1:
                                    for pp in range(NP):
                                        inter.append(("a", pp, qb + 1, 0, 0))
                                    for tt in range(4 * qb + 4, 4 * qb + 8):
                                        inter.append(("b", tt, 0, 0, 0))
                                if do_d and qb > 0:
                                    for tt in range(4 * (qb - 1), 4 * qb):
                                        inter.append(("d", tt, 0, 0, 0))
                                ng = 4 * (qb + 1)
                                glist = [
                                    ("g", qb, p, g, ng)
                                    for p in range(NP)
                                    for g in range(ng)
                                ]
                                if inter:
                                    step = max(1, len(glist) // len(inter))
                                    woven, ii = [], 0
                                    for j, t in enumerate(glist):
                                        woven.append(t)
                                        if j % step == step - 1 and ii < len(inter):
                                            woven.append(inter[ii])
                                            ii += 1
                                    woven += inter[ii:]
                                    glist = woven
                                tasks += glist
                            if do_d:
                                for tt in range(TT - 4, TT):
                                    tasks.append(("d", tt, 0, 0, 0))
                        elif do_d:
                            for tt in range(TT):
                                tasks.append(("d", tt, 0, 0, 0))

                        pt_store = {}
                        s_store = {}
                        y_store = {}

                        def emit_front(idx):
                            kind, qb, p, g, ng = tasks[idx]
                            if kind != "g":
                                return
                            r = g - 4 * qb  # >=0: diagonal-straddling chunk
                            lo = 128 * r if r > 0 else 0
                            ksl = slice(g * 128, (g + 1) * 128)
                            s = pss.tile([128, 2, 512], f32, tag="s",
                                         name=f"s_{idx}")
                            for hf in range(2):
                                nc.tensor.matmul(
                                    s[:, hf, lo:512],
                                    kt_sb[64 * hf: 64 * (hf + 1), p, ksl],
                                    qt_sb[64 * hf: 64 * (hf + 1), p,
                                          qb * 512 + lo: (qb + 1) * 512],
                                    start=True, stop=True,
                                )
                            pt = ptp.tile([128, 2, 512], bf16, tag="pt",
                                          name=f"pt_{idx}")
                            # one exp per group (strided AP when straddling)
                            nc.scalar.activation(
                                pt[:, :, lo:512], s[:, :, lo:512],
                                mybir.ActivationFunctionType.Exp,
                                scale=exp_scale,
                            )
                            if r >= 0:
                                # triangle mask on the diagonal 128-wide strip
                                for hf in range(2):
                                    nc.vector.tensor_mul(
                                        pt[:, hf, lo:lo + 128],
                                        pt[:, hf, lo:lo + 128], tri_sb[:])
                            pt_store[idx] = pt

                        def emit_back(idx):
                            kind, qb, p, g, ng = tasks[idx]
                            if kind == "a":
                                pp, tb = qb, p
                                tsl = slice(tb * 512, (tb + 1) * 512)
                                psl = slice(pp * 128, (pp + 1) * 128)
                                psq = psp.tile([128, 512], f32, tag="o",
                                               name=f"psq{pp}_{tb}")
                                if fp8a:
                                    for kp in range(4):
                                        nc.tensor.matmul(
                                            psq[:], wq8_sb[:, kp, :, psl],
                                            xt8_sb[:, 2 * kp:2 * kp + 2, tsl],
                                            start=(kp == 0), stop=(kp == 3),
                                            perf_mode=(
                                                mybir.MatmulPerfMode.DoubleRow),
                                        )
                                else:
                                    for k in range(CK):
                                        nc.tensor.matmul(
                                            psq[:], wq_sb[:, k, psl],
                                            xt_sb[:, k, tsl],
                                            start=(k == 0), stop=(k == CK - 1),
                                        )
                                nc.vector.tensor_copy(qt_sb[:, pp, tsl], psq[:])
                                psk = psp.tile([128, 512], f32, tag="o",
                                               name=f"psk{pp}_{tb}")
                                if fp8a:
                                    for kp in range(4):
                                        nc.tensor.matmul(
                                            psk[:], wk8_sb[:, kp, :, psl],
                                            xt8_sb[:, 2 * kp:2 * kp + 2, tsl],
                                            start=(kp == 0), stop=(kp == 3),
                                            perf_mode=(
                                                mybir.MatmulPerfMode.DoubleRow),
                                        )
                                else:
                                    for k in range(CK):
                                        nc.tensor.matmul(
                                            psk[:], wk_sb[:, k, psl],
                                            xt_sb[:, k, tsl],
                                            start=(k == 0), stop=(k == CK - 1),
                                        )
                                nc.vector.tensor_copy(kt_sb[:, pp, tsl], psk[:])
                                return
                            if kind == "b":
                                tt = qb
                                psv = psp.tile([128, 512], f32, tag="o",
                                               name=f"psv{tt}")
                                for k in range(CK):
                                    nc.tensor.matmul(
                                        psv[:],
                                        xt_sb[:, k, tt * 128: (tt + 1) * 128],
                                        wv_sb[:, k, :],
                                        start=(k == 0), stop=(k == CK - 1),
                                    )
                                nc.vector.tensor_copy(
                                    v_sb[:, tt, :, 0:HD],
                                    psv[:].rearrange("p (h e) -> p h e", e=HD),
                                )
                                return
                            if kind == "d":
                                tt = qb
                                tsl = slice(tt * 128, (tt + 1) * 128)
                                ot = outp.tile([128, 1024], f32, tag="ot")
                                for nb in range(2):
                                    po = psp.tile([128, 512], f32, tag="o",
                                                  name=f"po{tt}_{nb}")
                                    for pp2 in range(NP):
                                        nc.tensor.matmul(
                                            po[:],
                                            yt_sb[pp2][:, tsl],
                                            wp_sb[:, pp2,
                                                  nb * 512: (nb + 1) * 512],
                                            start=(pp2 == 0),
                                            stop=(pp2 == NP - 1),
                                        )
                                    nc.vector.tensor_copy(
                                        ot[:, nb * 512: (nb + 1) * 512], po[:])
                                nc.sync.dma_start(out_d[tsl, :], ot[:])
                                return
                            # kind == "g"
                            r = g - 4 * qb
                            lo = 128 * r if r > 0 else 0
                            pt = pt_store.pop(idx)
                            if (qb, p) not in y_store:
                                ya = psy.tile([65, 512], f32, tag="y",
                                              name=f"ya_{qb}_{p}")
                                yb = psy.tile([65, 512], f32, tag="y",
                                              name=f"yb_{qb}_{p}")
                                y_store[(qb, p)] = (ya, yb)
                            ya, yb = y_store[(qb, p)]
                            if do_av:
                                for hf, yy in ((0, ya), (1, yb)):
                                    nc.tensor.matmul(
                                        yy[:, lo:512],
                                        v_sb[:, g, 2 * p + hf, :],
                                        pt[:, hf, lo:512],
                                        start=(g == 0), stop=(g == ng - 1),
                                        skip_group_check=True,
                                    )
                            if g != ng - 1:
                                return
                            if not do_av:
                                nc.vector.memset(ya[:], 1.0)
                                nc.vector.memset(yb[:], 1.0)
                            # normalize: yt = y[0:64] * (1 / rowsum)
                            qsl = slice(qb * 512, (qb + 1) * 512)
                            for hi, yy in ((0, ya), (1, yb)):
                                rec = recp.tile([1, 512], f32, tag="rec",
                                                name=f"rec_{qb}_{p}_{hi}")
                                if norm == "gps":
                                    nc.vector.reciprocal_approx_fast(
                                        rec[0:1, :], yy[64:65, :])
                                else:  # "sbuf": stage rowsum in SBUF first
                                    rs = recp.tile([1, 512], f32, tag="rs",
                                                   name=f"rs_{qb}_{p}_{hi}")
                                    nc.vector.tensor_copy(
                                        rs[0:1, :], yy[64:65, :])
                                    nc.vector.reciprocal_approx_fast(
                                        rec[0:1, :], rs[0:1, :])
                                bc = bcp.tile([64, 512], f32, tag="bc",
                                              name=f"bc_{qb}_{p}_{hi}")
                                nc.gpsimd.partition_broadcast(
                                    bc[:], rec[0:1, :], channels=64)
                                nc.vector.tensor_mul(
                                    yt_sb[p][hi * 64: (hi + 1) * 64, qsl],
                                    yy[0:64, :], bc[:],
                                )
                            del y_store[(qb, p)]

                        n = len(tasks)
                        for j in range(min(la, n)):
                            emit_front(j)
                        for i in range(n):
                            if i + la < n:
                                emit_front(i + la)
                            emit_back(i)

                wqk_cm.__exit__(None, None, None)
                xtp_cm.__exit__(None, None, None)
                vp_cm.__exit__(None, None, None)

            if rep == 1:
                body()
            else:
                with tc.For_i(0, rep, 1):
                    body()

    nc.compile()
    return nc


@functools.lru_cache(maxsize=None)
def _get_nc(rep=1, la=5, sbufs=2, ybufs=2, pbufs=2, ptbufs=8, ablate="full",
            norm="sbuf", fp8a=False, xbufs=1, vbufs=1):
    return _build(rep, la, sbufs, ybufs, pbufs, ptbufs, ablate, norm, fp8a,
                  xbufs, vbufs)


FP8A = False  # must match the _build/_get_runner default


def make_in_maps(x, w_qkv, w_proj):
    import ml_dtypes
    bf16 = ml_dtypes.bfloat16
    j = np.arange(128)[None, :]
    i = np.arange(128)[:, None]
    tri = (j >= i).astype(bf16)

    in_maps = []
    for core in range(N_CORES):
        b, hg = divmod(core, HG)
        sl = slice(hg * 512, (hg + 1) * 512)
        xtb = np.ascontiguousarray(x[b].T)
        wqt = np.ascontiguousarray(w_qkv[sl].T)
        wkt = np.ascontiguousarray(w_qkv[C:2 * C][sl].T)
        m = {
            "xt": xtb.astype(bf16),
            "wq": wqt.astype(bf16),
            "wk": wkt.astype(bf16),
            "wv": np.ascontiguousarray(w_qkv[2 * C:3 * C][sl].T).astype(bf16),
            "wp": np.ascontiguousarray(w_proj[:, sl].T).astype(bf16),
            "tri": tri,
        }
        if FP8A:
            f8 = ml_dtypes.float8_e4m3

            def dr8(wt):
                # [C,512] -> DoubleRow-interleaved [128, kp*j*m] fp8, x32
                return np.ascontiguousarray(
                    (wt * 32.0).reshape(4, 2, 128, 512).transpose(2, 0, 1, 3)
                    .reshape(128, 4096)).astype(f8)

            m["xt8"] = xtb.astype(f8)
            m["wq8"] = dr8(wqt)
            m["wk8"] = dr8(wkt)
        in_maps.append(m)
    return in_maps


def combine(results):
    out = np.empty((B, T, C), dtype=np.float32)
    for b in range(B):
        out[b] = results[2 * b]["out"] + results[2 * b + 1]["out"]
    return out


# ---------------------------------------------------------------------------
# PJRT runner (device-resident inputs, reusable jitted executable)
# ---------------------------------------------------------------------------

class _Runner:
    def __init__(self, nc, n_cores=N_CORES):
        import jax
        import concourse.mybir as mybir
        from concourse import bass2jax
        from jax.sharding import Mesh, PartitionSpec, NamedSharding
        from jax.experimental.shard_map import shard_map

        self.jax = jax
        bass2jax.install_neuronx_cc_hook()
        partition_name = (
            nc.partition_id_tensor.name if nc.partition_id_tensor else None
        )
        in_names, out_names, out_avals, zero_outs = [], [], [], []
        for alloc in nc.m.functions[0].allocations:
            if not isinstance(alloc, mybir.MemoryLocationSet):
                continue
            name = alloc.memorylocations[0].name
            if alloc.kind == "ExternalInput":
                if name != partition_name:
                    in_names.append(name)
            elif alloc.kind == "ExternalOutput":
                out_names.append(name)
                shape = tuple(alloc.tensor_shape)
                dtype = mybir.dt.np(alloc.dtype)
                out_avals.append(jax.core.ShapedArray(shape, dtype))
                zero_outs.append(np.zeros(shape, dtype))
        self.in_names, self.out_names = in_names, out_names
        self.out_avals, self.zero_outs = out_avals, zero_outs
        self.n_cores = n_cores
        all_names = in_names + out_names
        if partition_name is not None:
            all_names = all_names + [partition_name]

        def _bdy(*args):
            operands = list(args)
            if partition_name is not None:
                operands.append(bass2jax.partition_id_tensor())
            outs = bass2jax._bass_exec_p.bind(
                *operands,
                out_avals=tuple(out_avals),
                in_names=tuple(all_names),
                out_names=tuple(out_names),
                lowering_input_output_aliases=(),
                sim_require_finite=True,
                sim_require_nnan=True,
                nc=nc,
            )
            return tuple(outs)

        devices = jax.devices()[:n_cores]
        mesh = Mesh(np.asarray(devices), ("core",))
        n_args = len(in_names) + len(out_names)
        self.fn = jax.jit(
            shard_map(
                _bdy, mesh=mesh,
                in_specs=(PartitionSpec("core"),) * n_args,
                out_specs=(PartitionSpec("core"),) * len(out_names),
                check_rep=False,
            ),
            keep_unused=True,
        )
        self.sharding = NamedSharding(mesh, PartitionSpec("core"))

    def put_inputs(self, in_maps):
        concat = [
            np.concatenate([np.asarray(m[name]) for m in in_maps], axis=0)
            for name in self.in_names
        ]
        concat += [
            np.zeros((self.n_cores * z.shape[0], *z.shape[1:]), z.dtype)
            for z in self.zero_outs
        ]
        self.args = [self.jax.device_put(a, self.sharding) for a in concat]
        self.jax.block_until_ready(self.args)

    def run(self):
        outs = self.fn(*self.args)
        self.jax.block_until_ready(outs)
        return [
            {
                name: np.asarray(outs[i]).reshape(
                    self.n_cores, *self.out_avals[i].shape)[c]
                for i, name in enumerate(self.out_names)
            }
            for c in range(self.n_cores)
        ]

    def time_ns(self, iters=20, warmup=2):
        import time
        for _ in range(warmup):
            self.jax.block_until_ready(self.fn(*self.args))
        t0 = time.perf_counter()
        outs = None
        for _ in range(iters):
            outs = self.fn(*self.args)
        self.jax.block_until_ready(outs)
        t1 = time.perf_counter()
        return (t1 - t0) / iters * 1e9



@functools.lru_cache(maxsize=None)
def _get_runner(rep=1, la=5, sbufs=2, ybufs=2, pbufs=2, ptbufs=8, ablate="full",
                norm="sbuf", fp8a=False, xbufs=1, vbufs=1):
    return _Runner(_get_nc(rep, la, sbufs, ybufs, pbufs, ptbufs, ablate, norm,
                           fp8a, xbufs, vbufs))


def kernel(x, w_qkv, w_proj):
    x = np.asarray(x, dtype=np.float32)
    w_qkv = np.asarray(w_qkv, dtype=np.float32)
    w_proj = np.asarray(w_proj, dtype=np.float32)
    runner = _get_runner()
    runner.put_inputs(make_in_maps(x, w_qkv, w_proj))
    return combine(runner.run())


# revision 37
# speedup vs baseline: 1.0335x; 1.0295x over previous
"""Causal self-attention v7 (B=4, T=2048, C=1024, H=16) on 8 trn2 NeuronCores.

Sharding: core = (batch b, head-group hg), b = core//2, hg = core%2 (Megatron
column-parallel qkv / row-parallel proj); host sums the two partial outputs.

Changes vs the 468us v2 baseline (measured ~283us, la=5/ptbufs=8):
  - qb-major schedule: proj tasks (a=QK, b=V, d=out-proj) woven INTO the
    attention group stream so the PE never idles while ACT (exp) works;
    d(qb-1) runs during C(qb).
  - normalization: DVE copy of the PSUM rowsum row (recip straight off PSUM
    is broken on HW - sim diverges), DVE reciprocal, GPSIMD
    partition_broadcast (idle engine) for the [64,512] broadcast, single DVE
    mul per head.  No more PE broadcast matmuls.
  - S/pt tiles are [128, 2, 512] so straddle groups get ONE exp over a
    strided AP instead of two (saves 352 ACT cycles per instruction).
  - proj PSUM tiles are one bank each ([128,512]) in their own pool, so an
    in-flight a/b/d task no longer blocks the S-tile pipeline.
  - PSUM budget: s 2x2 banks + y 2x1 + proj 2x1 = 8 banks.
  - yt stored per-pair (finer dep granularity for the tail d-tasks); v ones
    columns memset only (was a 7us whole-tile memset blocking DVE each rep).
  - batched DMAs (rearranged whole-tensor transfers, one out-DMA per
    t-tile): each dma_start costs ~600ns of HWDGE issue time.
  - fp8a (DoubleRow QK-projection) exists but is OFF: e4m3 q/k noise puts
    max-rel err at 3.5e-2 > 2e-2 tolerance.
  - lookahead la=5 with ptbufs=8 (exp/mask run ~5 groups ahead of the AV
    matmuls): rep3-trace sweep la=3/4/5/6 -> 844.8/833.8/827.6/832.6us.
"""

import functools

import numpy as np

B, T, C, H = 4, 2048, 1024, 16
HD = C // H  # 64
N_CORES = 8
HG = 2  # head groups
NH = H // HG  # heads per core = 8
NP = NH // 2  # head pairs per core = 4
TT = T // 128  # 16 t-tiles
TB = T // 512  # 4 t-blocks
CK = C // 128  # 8 c-chunks


def _build(rep=1, la=5, sbufs=2, ybufs=2, pbufs=2, ptbufs=8, ablate="full",
           norm="sbuf", fp8a=False, xbufs=1, vbufs=1):
    import concourse.bass as bass
    import concourse.mybir as mybir
    import concourse.tile as tile
    from concourse import bacc

    f32 = mybir.dt.float32
    bf16 = mybir.dt.bfloat16
    f8 = mybir.dt.float8e4

    nc = bacc.Bacc("TRN2", target_bir_lowering=False, debug=False)

    xt_d = nc.dram_tensor("xt", [C, T], bf16, kind="ExternalInput")
    if fp8a:
        # fp8 copies for the DoubleRow QK projection; wq8/wk8 are already
        # interleaved host-side as [ki=128, kpair=4, j=2, m=512] and carry a
        # x32 pre-scale (compensated in the exp scale) to clear the e4m3
        # subnormal range.
        xt8_d = nc.dram_tensor("xt8", [C, T], f8, kind="ExternalInput")
        wq8_d = nc.dram_tensor("wq8", [128, 4096], f8, kind="ExternalInput")
        wk8_d = nc.dram_tensor("wk8", [128, 4096], f8, kind="ExternalInput")
    else:
        wq_d = nc.dram_tensor("wq", [C, 512], bf16, kind="ExternalInput")
        wk_d = nc.dram_tensor("wk", [C, 512], bf16, kind="ExternalInput")
    wv_d = nc.dram_tensor("wv", [C, 512], bf16, kind="ExternalInput")
    wp_d = nc.dram_tensor("wp", [512, C], bf16, kind="ExternalInput")
    tri_d = nc.dram_tensor("tri", [128, 128], bf16, kind="ExternalInput")
    out_d = nc.dram_tensor("out", [T, C], f32, kind="ExternalOutput")

    exp_scale = 0.125 / 1024.0 if fp8a else 0.125

    do_attn = ablate in ("full", "noav")
    do_av = ablate in ("full",)
    do_d = ablate in ("full", "noattn")

    with tile.TileContext(nc) as tc:
        with tc.tile_pool(name="persist", bufs=1) as persist:
            qt_sb = persist.tile([128, NP, T], bf16, tag="qt")
            kt_sb = persist.tile([128, NP, T], bf16, tag="kt")

            def body():
                # strictly nested (LIFO) pool lifetimes
                vp_cm = tc.tile_pool(name="vp", bufs=vbufs)
                xtp_cm = tc.tile_pool(name="xtp", bufs=xbufs)
                wqk_cm = tc.tile_pool(name="wqk", bufs=1)
                vp = vp_cm.__enter__()
                xtp = xtp_cm.__enter__()
                wqk = wqk_cm.__enter__()

                v_sb = vp.tile([128, TT, NH, HD + 1], bf16, tag="v")
                # ones columns of V' only (V-proj copies fill 0:HD)
                nc.vector.memset(v_sb[:, :, :, HD:HD + 1], 1.0)

                if fp8a:
                    wq8_sb = wqk.tile([128, 4, 2, 512], f8, tag="wq8")
                    wk8_sb = wqk.tile([128, 4, 2, 512], f8, tag="wk8")
                    xt8_sb = xtp.tile([128, CK, T], f8, tag="xt8")
                else:
                    wq_sb = wqk.tile([128, CK, 512], bf16, tag="wq")
                    wk_sb = wqk.tile([128, CK, 512], bf16, tag="wk")
                wv_sb = wqk.tile([128, CK, 512], bf16, tag="wv")
                wp_sb = wqk.tile([128, NP, C], bf16, tag="wp")
                tri_sb = wqk.tile([128, 128], bf16, tag="tri")
                xt_sb = xtp.tile([128, CK, T], bf16, tag="xt")
                # exp-table preload on the idle ACT engine during the DMA head
                warm = wqk.tile([128, 32], bf16, tag="warm")
                nc.vector.memset(warm[:], 1.0)
                nc.scalar.activation(
                    warm[0:1, 16:32], warm[0:1, 0:16],
                    mybir.ActivationFunctionType.Exp, scale=exp_scale,
                )
                # chunked DMAs in consumption order; tri first (first-unit
                # masks); QK operands for tb=0 first so phase A starts early.
                nc.sync.dma_start(tri_sb[:], tri_d[:, :])
                if fp8a:
                    nc.sync.dma_start(wq8_sb[:], wq8_d.rearrange(
                        "p (a b n) -> p a b n", a=4, b=2))
                    nc.sync.dma_start(wk8_sb[:], wk8_d.rearrange(
                        "p (a b n) -> p a b n", a=4, b=2))
                    for tb in range(TB):
                        tsl = slice(tb * 512, (tb + 1) * 512)
                        for k in range(CK):
                            ksl = slice(k * 128, (k + 1) * 128)
                            nc.sync.dma_start(
                                xt8_sb[:, k, tsl], xt8_d[ksl, tsl])
                    for k in range(CK):
                        ksl = slice(k * 128, (k + 1) * 128)
                        nc.sync.dma_start(xt_sb[:, k, 0:512], xt_d[ksl, 0:512])
                        nc.sync.dma_start(wv_sb[:, k, :], wv_d[ksl, :])
                else:
                    nc.sync.dma_start(
                        wq_sb[:], wq_d.rearrange("(a p) n -> p a n", p=128))
                    nc.sync.dma_start(
                        xt_sb[:, :, 0:512],
                        xt_d[:, 0:512].rearrange("(a p) t -> p a t", p=128))
                    nc.sync.dma_start(
                        wk_sb[:], wk_d.rearrange("(a p) n -> p a n", p=128))
                    nc.sync.dma_start(
                        wv_sb[:], wv_d.rearrange("(a p) n -> p a n", p=128))
                for tb in range(1, TB):
                    tsl = slice(tb * 512, (tb + 1) * 512)
                    nc.sync.dma_start(
                        xt_sb[:, :, tsl],
                        xt_d[:, tsl].rearrange("(a p) t -> p a t", p=128))
                nc.sync.dma_start(wp_sb[:], wp_d.rearrange("(a p) n -> p a n", p=128))

                with (
                    tc.tile_pool(name="persist2", bufs=1) as persist2,
                    tc.tile_pool(name="ptp", bufs=ptbufs) as ptp,
                    tc.tile_pool(name="recp", bufs=2) as recp,
                    tc.tile_pool(name="bcp", bufs=2) as bcp,
                    tc.tile_pool(name="outp", bufs=4) as outp,
                ):
                    yt_sb = [
                        persist2.tile([128, T], bf16, tag=f"yt{p}",
                                      name=f"yt{p}")
                        for p in range(NP)
                    ]
                    if ablate in ("noattn",):
                        for p in range(NP):
                            nc.vector.memset(yt_sb[p][:], 0.001)
                    with (
                        tc.tile_pool(name="pss", bufs=sbufs, space="PSUM") as pss,
                        tc.tile_pool(name="psy", bufs=ybufs, space="PSUM") as psy,
                        tc.tile_pool(name="psp", bufs=pbufs, space="PSUM") as psp,
                    ):
                        # ---- task list: qb-major, proj tasks woven in
                        tasks = []
                        if do_attn:
                            for pp in range(NP):
                                tasks.append(("a", pp, 0, 0, 0))
                            for tt in range(4):
                                tasks.append(("b", tt, 0, 0, 0))
                            for qb in range(TB):
                                inter = []
                                if qb < TB - 1:
                                    for pp in range(NP):
                                        inter.append(("a", pp, qb + 1, 0, 0))
                                    for tt in range(4 * qb + 4, 4 * qb + 8):
                                        inter.append(("b", tt, 0, 0, 0))
                                if do_d and qb > 0:
                                    for tt in range(4 * (qb - 1), 4 * qb):
                                        inter.append(("d", tt, 0, 0, 0))
                                ng = 4 * (qb + 1)
                                glist = [
                                    ("g", qb, p, g, ng)
                                    for p in range(NP)
                                    for g in range(ng)
                                ]
                                if inter:
                                    step = max(1, len(glist) // len(inter))
                                    woven, ii = [], 0
                                    for j, t in enumerate(glist):
                                        woven.append(t)
                                        if j % step == step - 1 and ii < len(inter):
                                            woven.append(inter[ii])
                                            ii += 1
                                    woven += inter[ii:]
                                    glist = woven
                                tasks += glist
                            if do_d:
                                for tt in range(TT - 4, TT):
                                    tasks.append(("d", tt, 0, 0, 0))
                        elif do_d:
                            for tt in range(TT):
                                tasks.append(("d", tt, 0, 0, 0))

                        pt_store = {}
                        s_store = {}
                        y_store = {}

                        def emit_front(idx):
                            kind, qb, p, g, ng = tasks[idx]
                            if kind != "g":
                                return
                            r = g - 4 * qb  # >=0: diagonal-straddling chunk
                            lo = 128 * r if r > 0 else 0
                            ksl = slice(g * 128, (g + 1) * 128)
                            s = pss.tile([128, 2, 512], f32, tag="s",
                                         name=f"s_{idx}")
                            for hf in range(2):
                                nc.tensor.matmul(
                                    s[:, hf, lo:512],
                                    kt_sb[64 * hf: 64 * (hf + 1), p, ksl],
                                    qt_sb[64 * hf: 64 * (hf + 1), p,
                                          qb * 512 + lo: (qb + 1) * 512],
                                    start=True, stop=True,
                                )
                            pt = ptp.tile([128, 2, 512], bf16, tag="pt",
                                          name=f"pt_{idx}")
                            # one exp per group (strided AP when straddling)
                            nc.scalar.activation(
                                pt[:, :, lo:512], s[:, :, lo:512],
                                mybir.ActivationFunctionType.Exp,
                                scale=exp_scale,
                            )
                            if r >= 0:
                                # triangle mask on the diagonal 128-wide strip
                                for hf in range(2):
                                    nc.vector.tensor_mul(
                                        pt[:, hf, lo:lo + 128],
                                        pt[:, hf, lo:lo + 128], tri_sb[:])
                            pt_store[idx] = pt

                        def emit_back(idx):
                            kind, qb, p, g, ng = tasks[idx]
                            if kind == "a":
                                pp, tb = qb, p
                                tsl = slice(tb * 512, (tb + 1) * 512)
                                psl = slice(pp * 128, (pp + 1) * 128)
                                psq = psp.tile([128, 512], f32, tag="o",
                                               name=f"psq{pp}_{tb}")
                                if fp8a:
                                    for kp in range(4):
                                        nc.tensor.matmul(
                                            psq[:], wq8_sb[:, kp, :, psl],
                                            xt8_sb[:, 2 * kp:2 * kp + 2, tsl],
                                            start=(kp == 0), stop=(kp == 3),
                                            perf_mode=(
                                                mybir.MatmulPerfMode.DoubleRow),
                                        )
                                else:
                                    for k in range(CK):
                                        nc.tensor.matmul(
                                            psq[:], wq_sb[:, k, psl],
                                            xt_sb[:, k, tsl],
                                            start=(k == 0), stop=(k == CK - 1),
                                        )
                                nc.vector.tensor_copy(qt_sb[:, pp, tsl], psq[:])
                                psk = psp.tile([128, 512], f32, tag="o",
                                               name=f"psk{pp}_{tb}")
                                if fp8a:
                                    for kp in range(4):
                                        nc.tensor.matmul(
                                            psk[:], wk8_sb[:, kp, :, psl],
                                            xt8_sb[:, 2 * kp:2 * kp + 2, tsl],
                                            start=(kp == 0), stop=(kp == 3),
                                            perf_mode=(
                                                mybir.MatmulPerfMode.DoubleRow),
                                        )
                                else:
                                    for k in range(CK):
                                        nc.tensor.matmul(
                                            psk[:], wk_sb[:, k, psl],
                                            xt_sb[:, k, tsl],
                                            start=(k == 0), stop=(k == CK - 1),
                                        )
                                nc.vector.tensor_copy(kt_sb[:, pp, tsl], psk[:])
                                return
                            if kind == "b":
                                tt = qb
                                psv = psp.tile([128, 512], f32, tag="o",
                                               name=f"psv{tt}")
                                for k in range(CK):
                                    nc.tensor.matmul(
                                        psv[:],
                                        xt_sb[:, k, tt * 128: (tt + 1) * 128],
                                        wv_sb[:, k, :],
                                        start=(k == 0), stop=(k == CK - 1),
                                    )
                                nc.vector.tensor_copy(
                                    v_sb[:, tt, :, 0:HD],
                                    psv[:].rearrange("p (h e) -> p h e", e=HD),
                                )
                                return
                            if kind == "d":
                                tt = qb
                                tsl = slice(tt * 128, (tt + 1) * 128)
                                ot = outp.tile([128, 1024], f32, tag="ot")
                                for nb in range(2):
                                    po = psp.tile([128, 512], f32, tag="o",
                                                  name=f"po{tt}_{nb}")
                                    for pp2 in range(NP):
                                        nc.tensor.matmul(
                                            po[:],
                                            yt_sb[pp2][:, tsl],
                                            wp_sb[:, pp2,
                                                  nb * 512: (nb + 1) * 512],
                                            start=(pp2 == 0),
                                            stop=(pp2 == NP - 1),
                                        )
                                    nc.vector.tensor_copy(
                                        ot[:, nb * 512: (nb + 1) * 512], po[:])
                                nc.sync.dma_start(out_d[tsl, :], ot[:])
                                return
                            # kind == "g"
                            r = g - 4 * qb
                            lo = 128 * r if r > 0 else 0
                            pt = pt_store.pop(idx)
                            if (qb, p) not in y_store:
                                ya = psy.tile([65, 512], f32, tag="y",
                                              name=f"ya_{qb}_{p}")
                                yb = psy.tile([65, 512], f32, tag="y",
                                              name=f"yb_{qb}_{p}")
                                y_store[(qb, p)] = (ya, yb)
                            ya, yb = y_store[(qb, p)]
                            if do_av:
                                for hf, yy in ((0, ya), (1, yb)):
                                    nc.tensor.matmul(
                                        yy[:, lo:512],
                                        v_sb[:, g, 2 * p + hf, :],
                                        pt[:, hf, lo:512],
                                        start=(g == 0), stop=(g == ng - 1),
                                        skip_group_check=True,
                                    )
                            if g != ng - 1:
                                return
                            if not do_av:
                                nc.vector.memset(ya[:], 1.0)
                                nc.vector.memset(yb[:], 1.0)
                            # normalize: yt = y[0:64] * (1 / rowsum)
                            qsl = slice(qb * 512, (qb + 1) * 512)
                            for hi, yy in ((0, ya), (1, yb)):
                                rec = recp.tile([1, 512], f32, tag="rec",
                                                name=f"rec_{qb}_{p}_{hi}")
                                if norm == "gps":
                                    nc.vector.reciprocal_approx_fast(
                                        rec[0:1, :], yy[64:65, :])
                                else:  # "sbuf": stage rowsum in SBUF first
                                    rs = recp.tile([1, 512], f32, tag="rs",
                                                   name=f"rs_{qb}_{p}_{hi}")
                                    nc.vector.tensor_copy(
                                        rs[0:1, :], yy[64:65, :])
                                    nc.vector.reciprocal_approx_fast(
                                        rec[0:1, :], rs[0:1, :])
                                bc = bcp.tile([64, 512], f32, tag="bc",
                                              name=f"bc_{qb}_{p}_{hi}")
                                nc.gpsimd.partition_broadcast(
                                    bc[:], rec[0:1, :], channels=64)
                                nc.vector.tensor_mul(
                                    yt_sb[p][hi * 64: (hi + 1) * 64, qsl],
                                    yy[0:64, :], bc[:],
                                )
                            del y_store[(qb, p)]

                        n = len(tasks)
                        for j in range(min(la, n)):
                            emit_front(j)
                        for i in range(n):
                            if i + la < n:
                                emit_front(i + la)
                            emit_back(i)

                wqk_cm.__exit__(None, None, None)
                xtp_cm.__exit__(None, None, None)
                vp_cm.__exit__(None, None, None)

            if rep == 1:
                body()
            else:
                with tc.For_i(0, rep, 1):
                    body()

    nc.compile()
    return nc


@functools.lru_cache(maxsize=None)
def _get_nc(rep=1, la=5, sbufs=2, ybufs=2, pbufs=2, ptbufs=8, ablate="full",
            norm="sbuf", fp8a=False, xbufs=1, vbufs=1):
    return _build(rep, la, sbufs, ybufs, pbufs, ptbufs, ablate, norm, fp8a,
                  xbufs, vbufs)


FP8A = False  # must match the _build/_get_runner default


def make_in_maps(x, w_qkv, w_proj):
    import ml_dtypes
    bf16 = ml_dtypes.bfloat16
    j = np.arange(128)[None, :]
    i = np.arange(128)[:, None]
    tri = (j >= i).astype(bf16)

    in_maps = []
    for core in range(N_CORES):
        b, hg = divmod(core, HG)
        sl = slice(hg * 512, (hg + 1) * 512)
        xtb = np.ascontiguousarray(x[b].T)
        wqt = np.ascontiguousarray(w_qkv[sl].T)
        wkt = np.ascontiguousarray(w_qkv[C:2 * C][sl].T)
        m = {
            "xt": xtb.astype(bf16),
            "wq": wqt.astype(bf16),
            "wk": wkt.astype(bf16),
            "wv": np.ascontiguousarray(w_qkv[2 * C:3 * C][sl].T).astype(bf16),
            "wp": np.ascontiguousarray(w_proj[:, sl].T).astype(bf16),
            "tri": tri,
        }
        if FP8A:
            f8 = ml_dtypes.float8_e4m3

            def dr8(wt):
                # [C,512] -> DoubleRow-interleaved [128, kp*j*m] fp8, x32
                return np.ascontiguousarray(
                    (wt * 32.0).reshape(4, 2, 128, 512).transpose(2, 0, 1, 3)
                    .reshape(128, 4096)).astype(f8)

            m["xt8"] = xtb.astype(f8)
            m["wq8"] = dr8(wqt)
            m["wk8"] = dr8(wkt)
        in_maps.append(m)
    return in_maps


def combine(results):
    out = np.empty((B, T, C), dtype=np.float32)
    for b in range(B):
        out[b] = results[2 * b]["out"] + results[2 * b + 1]["out"]
    return out


# ---------------------------------------------------------------------------
# PJRT runner (device-resident inputs, reusable jitted executable)
# ---------------------------------------------------------------------------

class _Runner:
    def __init__(self, nc, n_cores=N_CORES):
        import jax
        import concourse.mybir as mybir
        from concourse import bass2jax
        from jax.sharding import Mesh, PartitionSpec, NamedSharding
        from jax.experimental.shard_map import shard_map

        self.jax = jax
        bass2jax.install_neuronx_cc_hook()
        partition_name = (
            nc.partition_id_tensor.name if nc.partition_id_tensor else None
        )
        in_names, out_names, out_avals, zero_outs = [], [], [], []
        for alloc in nc.m.functions[0].allocations:
            if not isinstance(alloc, mybir.MemoryLocationSet):
                continue
            name = alloc.memorylocations[0].name
            if alloc.kind == "ExternalInput":
                if name != partition_name:
                    in_names.append(name)
            elif alloc.kind == "ExternalOutput":
                out_names.append(name)
                shape = tuple(alloc.tensor_shape)
                dtype = mybir.dt.np(alloc.dtype)
                out_avals.append(jax.core.ShapedArray(shape, dtype))
                zero_outs.append(np.zeros(shape, dtype))
        self.in_names, self.out_names = in_names, out_names
        self.out_avals, self.zero_outs = out_avals, zero_outs
        self.n_cores = n_cores
        all_names = in_names + out_names
        if partition_name is not None:
            all_names = all_names + [partition_name]

        def _bdy(*args):
            operands = list(args)
            if partition_name is not None:
                operands.append(bass2jax.partition_id_tensor())
            outs = bass2jax._bass_exec_p.bind(
                *operands,
                out_avals=tuple(out_avals),
                in_names=tuple(all_names),
                out_names=tuple(out_names),
                lowering_input_output_aliases=(),
                sim_require_finite=True,
                sim_require_nnan=True,
                nc=nc,
            )
            return tuple(outs)

        devices = jax.devices()[:n_cores]
        mesh = Mesh(np.asarray(devices), ("core",))
        n_args = len(in_names) + len(out_names)
        self.fn = jax.jit(
            shard_map(
                _bdy, mesh=mesh,
                in_specs=(PartitionSpec("core"),) * n_args,
                out_specs=(PartitionSpec("core"),) * len(out_names),
                check_rep=False,
            ),
            keep_unused=True,
        )
        self.sharding = NamedSharding(mesh, PartitionSpec("core"))

    def put_inputs(self, in_maps):
        concat = [
            np.concatenate([np.asarray(m[name]) for m in in_maps], axis=0)
            for name in self.in_names
        ]
        concat += [
            np.zeros((self.n_cores * z.shape[0], *z.shape[1:]), z.dtype)
            for z in self.zero_outs
        ]
        self.args = [self.jax.device_put(a, self.sharding) for a in concat]
        self.jax.block_until_ready(self.args)

    def run(self):
        outs = self.fn(*self.args)
        self.jax.block_until_ready(outs)
        return [
            {
                name: np.asarray(outs[i]).reshape(
                    self.n_cores, *self.out_avals[i].shape)[c]
                for i, name in enumerate(self.out_names)
            }
            for c in range(self.n_cores)
        ]

    def time_ns(self, iters=20, warmup=2):
        import time
        for _ in range(warmup):
            self.jax.block_until_ready(self.fn(*self.args))
        t0 = time.perf_counter()
        outs = None
        for _ in range(iters):
            outs = self.fn(*self.args)
        self.jax.block_until_ready(outs)
        t1 = time.perf_counter()
        return (t1 - t0) / iters * 1e9



@functools.lru_cache(maxsize=None)
def _get_runner(rep=1, la=5, sbufs=2, ybufs=2, pbufs=2, ptbufs=8, ablate="full",
                norm="sbuf", fp8a=False, xbufs=1, vbufs=1):
    return _Runner(_get_nc(rep, la, sbufs, ybufs, pbufs, ptbufs, ablate, norm,
                           fp8a, xbufs, vbufs))


def kernel(x, w_qkv, w_proj):
    x = np.asarray(x, dtype=np.float32)
    w_qkv = np.asarray(w_qkv, dtype=np.float32)
    w_proj = np.asarray(w_proj, dtype=np.float32)
    runner = _get_runner()
    runner.put_inputs(make_in_maps(x, w_qkv, w_proj))
    return combine(runner.run())


# revision 42
# speedup vs baseline: 1.0424x; 1.0087x over previous
"""Causal self-attention v7 (B=4, T=2048, C=1024, H=16) on 8 trn2 NeuronCores.

Sharding: core = (batch b, head-group hg), b = core//2, hg = core%2 (Megatron
column-parallel qkv / row-parallel proj); host sums the two partial outputs.

Changes vs the 468us v2 baseline (measured ~283-289us):
  - qb-major schedule: proj tasks (a=QK, b=V, d=out-proj) woven INTO the
    attention group stream so the PE never idles while ACT (exp) works;
    d(qb-1) runs during C(qb).
  - normalization: DVE copy of the PSUM rowsum row (recip straight off PSUM
    is broken on HW - sim diverges), DVE reciprocal, GPSIMD
    partition_broadcast (idle engine) for the [64,512] broadcast, single DVE
    mul per head.  No more PE broadcast matmuls.
  - S/pt tiles are [128, 2, 512] so straddle groups get ONE exp over a
    strided AP instead of two (saves 352 ACT cycles per instruction).
  - proj PSUM tiles are one bank each ([128,512]) in their own pool, so an
    in-flight a/b/d task no longer blocks the S-tile pipeline.
  - PSUM budget: s 2x2 banks + y 2x1 + proj 2x1 = 8 banks.
  - yt stored per-pair (finer dep granularity for the tail d-tasks); v ones
    columns memset only (was a 7us whole-tile memset blocking DVE each rep).
  - batched DMAs (rearranged whole-tensor transfers, one out-DMA per
    t-tile): each dma_start costs ~600ns of HWDGE issue time.
  - fp8a (DoubleRow QK-projection) exists but is OFF: e4m3 q/k noise puts
    max-rel err at 3.5e-2 > 2e-2 tolerance.
  - lookahead la=5 with ptbufs=8 (exp/mask run ~5 groups ahead of the AV
    matmuls): rep3-trace sweep la=3/4/5/6 -> 844.8/833.8/827.6/832.6us.
  - DMA order wq, xt(tb0), wk: the first a-task's q-half matmuls start as
    soon as wq+xt0 land; wk rides behind (rep3 trace 827.6 -> 818.6us).
"""

import functools

import numpy as np

B, T, C, H = 4, 2048, 1024, 16
HD = C // H  # 64
N_CORES = 8
HG = 2  # head groups
NH = H // HG  # heads per core = 8
NP = NH // 2  # head pairs per core = 4
TT = T // 128  # 16 t-tiles
TB = T // 512  # 4 t-blocks
CK = C // 128  # 8 c-chunks


def _build(rep=1, la=5, sbufs=2, ybufs=2, pbufs=2, ptbufs=8, ablate="full",
           norm="sbuf", fp8a=False, xbufs=1, vbufs=1):
    import concourse.bass as bass
    import concourse.mybir as mybir
    import concourse.tile as tile
    from concourse import bacc

    f32 = mybir.dt.float32
    bf16 = mybir.dt.bfloat16
    f8 = mybir.dt.float8e4

    nc = bacc.Bacc("TRN2", target_bir_lowering=False, debug=False)

    xt_d = nc.dram_tensor("xt", [C, T], bf16, kind="ExternalInput")
    if fp8a:
        # fp8 copies for the DoubleRow QK projection; wq8/wk8 are already
        # interleaved host-side as [ki=128, kpair=4, j=2, m=512] and carry a
        # x32 pre-scale (compensated in the exp scale) to clear the e4m3
        # subnormal range.
        xt8_d = nc.dram_tensor("xt8", [C, T], f8, kind="ExternalInput")
        wq8_d = nc.dram_tensor("wq8", [128, 4096], f8, kind="ExternalInput")
        wk8_d = nc.dram_tensor("wk8", [128, 4096], f8, kind="ExternalInput")
    else:
        wq_d = nc.dram_tensor("wq", [C, 512], bf16, kind="ExternalInput")
        wk_d = nc.dram_tensor("wk", [C, 512], bf16, kind="ExternalInput")
    wv_d = nc.dram_tensor("wv", [C, 512], bf16, kind="ExternalInput")
    wp_d = nc.dram_tensor("wp", [512, C], bf16, kind="ExternalInput")
    tri_d = nc.dram_tensor("tri", [128, 128], bf16, kind="ExternalInput")
    out_d = nc.dram_tensor("out", [T, C], f32, kind="ExternalOutput")

    exp_scale = 0.125 / 1024.0 if fp8a else 0.125

    do_attn = ablate in ("full", "noav")
    do_av = ablate in ("full",)
    do_d = ablate in ("full", "noattn")

    with tile.TileContext(nc) as tc:
        with tc.tile_pool(name="persist", bufs=1) as persist:
            qt_sb = persist.tile([128, NP, T], bf16, tag="qt")
            kt_sb = persist.tile([128, NP, T], bf16, tag="kt")

            def body():
                # strictly nested (LIFO) pool lifetimes
                vp_cm = tc.tile_pool(name="vp", bufs=vbufs)
                xtp_cm = tc.tile_pool(name="xtp", bufs=xbufs)
                wqk_cm = tc.tile_pool(name="wqk", bufs=1)
                vp = vp_cm.__enter__()
                xtp = xtp_cm.__enter__()
                wqk = wqk_cm.__enter__()

                v_sb = vp.tile([128, TT, NH, HD + 1], bf16, tag="v")
                # ones columns of V' only (V-proj copies fill 0:HD)
                nc.vector.memset(v_sb[:, :, :, HD:HD + 1], 1.0)

                if fp8a:
                    wq8_sb = wqk.tile([128, 4, 2, 512], f8, tag="wq8")
                    wk8_sb = wqk.tile([128, 4, 2, 512], f8, tag="wk8")
                    xt8_sb = xtp.tile([128, CK, T], f8, tag="xt8")
                else:
                    wq_sb = wqk.tile([128, CK, 512], bf16, tag="wq")
                    wk_sb = wqk.tile([128, CK, 512], bf16, tag="wk")
                wv_sb = wqk.tile([128, CK, 512], bf16, tag="wv")
                wp_sb = wqk.tile([128, NP, C], bf16, tag="wp")
                tri_sb = wqk.tile([128, 128], bf16, tag="tri")
                xt_sb = xtp.tile([128, CK, T], bf16, tag="xt")
                # exp-table preload on the idle ACT engine during the DMA head
                warm = wqk.tile([128, 32], bf16, tag="warm")
                nc.vector.memset(warm[:], 1.0)
                nc.scalar.activation(
                    warm[0:1, 16:32], warm[0:1, 0:16],
                    mybir.ActivationFunctionType.Exp, scale=exp_scale,
                )
                # chunked DMAs in consumption order; tri first (first-unit
                # masks); QK operands for tb=0 first so phase A starts early.
                nc.sync.dma_start(tri_sb[:], tri_d[:, :])
                if fp8a:
                    nc.sync.dma_start(wq8_sb[:], wq8_d.rearrange(
                        "p (a b n) -> p a b n", a=4, b=2))
                    nc.sync.dma_start(wk8_sb[:], wk8_d.rearrange(
                        "p (a b n) -> p a b n", a=4, b=2))
                    for tb in range(TB):
                        tsl = slice(tb * 512, (tb + 1) * 512)
                        for k in range(CK):
                            ksl = slice(k * 128, (k + 1) * 128)
                            nc.sync.dma_start(
                                xt8_sb[:, k, tsl], xt8_d[ksl, tsl])
                    for k in range(CK):
                        ksl = slice(k * 128, (k + 1) * 128)
                        nc.sync.dma_start(xt_sb[:, k, 0:512], xt_d[ksl, 0:512])
                        nc.sync.dma_start(wv_sb[:, k, :], wv_d[ksl, :])
                else:
                    nc.sync.dma_start(
                        wq_sb[:], wq_d.rearrange("(a p) n -> p a n", p=128))
                    nc.sync.dma_start(
                        xt_sb[:, :, 0:512],
                        xt_d[:, 0:512].rearrange("(a p) t -> p a t", p=128))
                    nc.sync.dma_start(
                        wk_sb[:], wk_d.rearrange("(a p) n -> p a n", p=128))
                    nc.sync.dma_start(
                        wv_sb[:], wv_d.rearrange("(a p) n -> p a n", p=128))
                for tb in range(1, TB):
                    tsl = slice(tb * 512, (tb + 1) * 512)
                    nc.sync.dma_start(
                        xt_sb[:, :, tsl],
                        xt_d[:, tsl].rearrange("(a p) t -> p a t", p=128))
                nc.sync.dma_start(wp_sb[:], wp_d.rearrange("(a p) n -> p a n", p=128))

                with (
                    tc.tile_pool(name="persist2", bufs=1) as persist2,
                    tc.tile_pool(name="ptp", bufs=ptbufs) as ptp,
                    tc.tile_pool(name="recp", bufs=2) as recp,
                    tc.tile_pool(name="bcp", bufs=2) as bcp,
                    tc.tile_pool(name="outp", bufs=4) as outp,
                ):
                    yt_sb = [
                        persist2.tile([128, T], bf16, tag=f"yt{p}",
                                      name=f"yt{p}")
                        for p in range(NP)
                    ]
                    if ablate in ("noattn",):
                        for p in range(NP):
                            nc.vector.memset(yt_sb[p][:], 0.001)
                    with (
                        tc.tile_pool(name="pss", bufs=sbufs, space="PSUM") as pss,
                        tc.tile_pool(name="psy", bufs=ybufs, space="PSUM") as psy,
                        tc.tile_pool(name="psp", bufs=pbufs, space="PSUM") as psp,
                    ):
                        # ---- task list: qb-major, proj tasks woven in
                        tasks = []
                        if do_attn:
                            for pp in range(NP):
                                tasks.append(("a", pp, 0, 0, 0))
                            for tt in range(4):
                                tasks.append(("b", tt, 0, 0, 0))
                            for qb in range(TB):
                                inter = []
                                if qb < TB - 1:
                                    for pp in range(NP):
                                        inter.append(("a", pp, qb + 1, 0, 0))
                                    for tt in range(4 * qb + 4, 4 * qb + 8):
                                        inter.append(("b", tt, 0, 0, 0))
                                if do_d and qb > 0:
                                    for tt in range(4 * (qb - 1), 4 * qb):
                                        inter.append(("d", tt, 0, 0, 0))
                                ng = 4 * (qb + 1)
                                glist = [
                                    ("g", qb, p, g, ng)
                                    for p in range(NP)
                                    for g in range(ng)
                                ]
                                if inter:
                                    step = max(1, len(glist) // len(inter))
                                    woven, ii = [], 0
                                    for j, t in enumerate(glist):
                                        woven.append(t)
                                        if j % step == step - 1 and ii < len(inter):
                                            woven.append(inter[ii])
                                            ii += 1
                                    woven += inter[ii:]
                                    glist = woven
                                tasks += glist
                            if do_d:
                                for tt in range(TT - 4, TT):
                                    tasks.append(("d", tt, 0, 0, 0))
                        elif do_d:
                            for tt in range(TT):
                                tasks.append(("d", tt, 0, 0, 0))

                        pt_store = {}
                        s_store = {}
                        y_store = {}

                        def emit_front(idx):
                            kind, qb, p, g, ng = tasks[idx]
                            if kind != "g":
                                return
                            r = g - 4 * qb  # >=0: diagonal-straddling chunk
                            lo = 128 * r if r > 0 else 0
                            ksl = slice(g * 128, (g + 1) * 128)
                            s = pss.tile([128, 2, 512], f32, tag="s",
                                         name=f"s_{idx}")
                            for hf in range(2):
                                nc.tensor.matmul(
                                    s[:, hf, lo:512],
                                    kt_sb[64 * hf: 64 * (hf + 1), p, ksl],
                                    qt_sb[64 * hf: 64 * (hf + 1), p,
                                          qb * 512 + lo: (qb + 1) * 512],
                                    start=True, stop=True,
                                )
                            pt = ptp.tile([128, 2, 512], bf16, tag="pt",
                                          name=f"pt_{idx}")
                            # one exp per group (strided AP when straddling)
                            nc.scalar.activation(
                                pt[:, :, lo:512], s[:, :, lo:512],
                                mybir.ActivationFunctionType.Exp,
                                scale=exp_scale,
                            )
                            if r >= 0:
                                # triangle mask on the diagonal 128-wide strip
                                for hf in range(2):
                                    nc.vector.tensor_mul(
                                        pt[:, hf, lo:lo + 128],
                                        pt[:, hf, lo:lo + 128], tri_sb[:])
                            pt_store[idx] = pt

                        def emit_back(idx):
                            kind, qb, p, g, ng = tasks[idx]
                            if kind == "a":
                                pp, tb = qb, p
                                tsl = slice(tb * 512, (tb + 1) * 512)
                                psl = slice(pp * 128, (pp + 1) * 128)
                                psq = psp.tile([128, 512], f32, tag="o",
                                               name=f"psq{pp}_{tb}")
                                if fp8a:
                                    for kp in range(4):
                                        nc.tensor.matmul(
                                            psq[:], wq8_sb[:, kp, :, psl],
                                            xt8_sb[:, 2 * kp:2 * kp + 2, tsl],
                                            start=(kp == 0), stop=(kp == 3),
                                            perf_mode=(
                                                mybir.MatmulPerfMode.DoubleRow),
                                        )
                                else:
                                    for k in range(CK):
                                        nc.tensor.matmul(
                                            psq[:], wq_sb[:, k, psl],
                                            xt_sb[:, k, tsl],
                                            start=(k == 0), stop=(k == CK - 1),
                                        )
                                nc.vector.tensor_copy(qt_sb[:, pp, tsl], psq[:])
                                psk = psp.tile([128, 512], f32, tag="o",
                                               name=f"psk{pp}_{tb}")
                                if fp8a:
                                    for kp in range(4):
                                        nc.tensor.matmul(
                                            psk[:], wk8_sb[:, kp, :, psl],
                                            xt8_sb[:, 2 * kp:2 * kp + 2, tsl],
                                            start=(kp == 0), stop=(kp == 3),
                                            perf_mode=(
                                                mybir.MatmulPerfMode.DoubleRow),
                                        )
                                else:
                                    for k in range(CK):
                                        nc.tensor.matmul(
                                            psk[:], wk_sb[:, k, psl],
                                            xt_sb[:, k, tsl],
                                            start=(k == 0), stop=(k == CK - 1),
                                        )
                                nc.vector.tensor_copy(kt_sb[:, pp, tsl], psk[:])
                                return
                            if kind == "b":
                                tt = qb
                                psv = psp.tile([128, 512], f32, tag="o",
                                               name=f"psv{tt}")
                                for k in range(CK):
                                    nc.tensor.matmul(
                                        psv[:],
                                        xt_sb[:, k, tt * 128: (tt + 1) * 128],
                                        wv_sb[:, k, :],
                                        start=(k == 0), stop=(k == CK - 1),
                                    )
                                nc.vector.tensor_copy(
                                    v_sb[:, tt, :, 0:HD],
                                    psv[:].rearrange("p (h e) -> p h e", e=HD),
                                )
                                return
                            if kind == "d":
                                tt = qb
                                tsl = slice(tt * 128, (tt + 1) * 128)
                                ot = outp.tile([128, 1024], f32, tag="ot")
                                for nb in range(2):
                                    po = psp.tile([128, 512], f32, tag="o",
                                                  name=f"po{tt}_{nb}")
                                    for pp2 in range(NP):
                                        nc.tensor.matmul(
                                            po[:],
                                            yt_sb[pp2][:, tsl],
                                            wp_sb[:, pp2,
                                                  nb * 512: (nb + 1) * 512],
                                            start=(pp2 == 0),
                                            stop=(pp2 == NP - 1),
                                        )
                                    nc.vector.tensor_copy(
                                        ot[:, nb * 512: (nb + 1) * 512], po[:])
                                nc.sync.dma_start(out_d[tsl, :], ot[:])
                                return
                            # kind == "g"
                            r = g - 4 * qb
                            lo = 128 * r if r > 0 else 0
                            pt = pt_store.pop(idx)
                            if (qb, p) not in y_store:
                                ya = psy.tile([65, 512], f32, tag="y",
                                              name=f"ya_{qb}_{p}")
                                yb = psy.tile([65, 512], f32, tag="y",
                                              name=f"yb_{qb}_{p}")
                                y_store[(qb, p)] = (ya, yb)
                            ya, yb = y_store[(qb, p)]
                            if do_av:
                                for hf, yy in ((0, ya), (1, yb)):
                                    nc.tensor.matmul(
                                        yy[:, lo:512],
                                        v_sb[:, g, 2 * p + hf, :],
                                        pt[:, hf, lo:512],
                                        start=(g == 0), stop=(g == ng - 1),
                                        skip_group_check=True,
                                    )
                            if g != ng - 1:
                                return
                            if not do_av:
                                nc.vector.memset(ya[:], 1.0)
                                nc.vector.memset(yb[:], 1.0)
                            # normalize: yt = y[0:64] * (1 / rowsum)
                            qsl = slice(qb * 512, (qb + 1) * 512)
                            for hi, yy in ((0, ya), (1, yb)):
                                rec = recp.tile([1, 512], f32, tag="rec",
                                                name=f"rec_{qb}_{p}_{hi}")
                                if norm == "gps":
                                    nc.vector.reciprocal_approx_fast(
                                        rec[0:1, :], yy[64:65, :])
                                else:  # "sbuf": stage rowsum in SBUF first
                                    rs = recp.tile([1, 512], f32, tag="rs",
                                                   name=f"rs_{qb}_{p}_{hi}")
                                    nc.vector.tensor_copy(
                                        rs[0:1, :], yy[64:65, :])
                                    nc.vector.reciprocal_approx_fast(
                                        rec[0:1, :], rs[0:1, :])
                                bc = bcp.tile([64, 512], f32, tag="bc",
                                              name=f"bc_{qb}_{p}_{hi}")
                                nc.gpsimd.partition_broadcast(
                                    bc[:], rec[0:1, :], channels=64)
                                nc.vector.tensor_mul(
                                    yt_sb[p][hi * 64: (hi + 1) * 64, qsl],
                                    yy[0:64, :], bc[:],
                                )
                            del y_store[(qb, p)]

                        n = len(tasks)
                        for j in range(min(la, n)):
                            emit_front(j)
                        for i in range(n):
                            if i + la < n:
                                emit_front(i + la)
                            emit_back(i)

                wqk_cm.__exit__(None, None, None)
                xtp_cm.__exit__(None, None, None)
                vp_cm.__exit__(None, None, None)

            if rep == 1:
                body()
            else:
                with tc.For_i(0, rep, 1):
                    body()

    nc.compile()
    return nc


@functools.lru_cache(maxsize=None)
def _get_nc(rep=1, la=5, sbufs=2, ybufs=2, pbufs=2, ptbufs=8, ablate="full",
            norm="sbuf", fp8a=False, xbufs=1, vbufs=1):
    return _build(rep, la, sbufs, ybufs, pbufs, ptbufs, ablate, norm, fp8a,
                  xbufs, vbufs)


FP8A = False  # must match the _build/_get_runner default


def make_in_maps(x, w_qkv, w_proj):
    import ml_dtypes
    bf16 = ml_dtypes.bfloat16
    j = np.arange(128)[None, :]
    i = np.arange(128)[:, None]
    tri = (j >= i).astype(bf16)

    in_maps = []
    for core in range(N_CORES):
        b, hg = divmod(core, HG)
        sl = slice(hg * 512, (hg + 1) * 512)
        xtb = np.ascontiguousarray(x[b].T)
        wqt = np.ascontiguousarray(w_qkv[sl].T)
        wkt = np.ascontiguousarray(w_qkv[C:2 * C][sl].T)
        m = {
            "xt": xtb.astype(bf16),
            "wq": wqt.astype(bf16),
            "wk": wkt.astype(bf16),
            "wv": np.ascontiguousarray(w_qkv[2 * C:3 * C][sl].T).astype(bf16),
            "wp": np.ascontiguousarray(w_proj[:, sl].T).astype(bf16),
            "tri": tri,
        }
        if FP8A:
            f8 = ml_dtypes.float8_e4m3

            def dr8(wt):
                # [C,512] -> DoubleRow-interleaved [128, kp*j*m] fp8, x32
                return np.ascontiguousarray(
                    (wt * 32.0).reshape(4, 2, 128, 512).transpose(2, 0, 1, 3)
                    .reshape(128, 4096)).astype(f8)

            m["xt8"] = xtb.astype(f8)
            m["wq8"] = dr8(wqt)
            m["wk8"] = dr8(wkt)
        in_maps.append(m)
    return in_maps


def combine(results):
    out = np.empty((B, T, C), dtype=np.float32)
    for b in range(B):
        out[b] = results[2 * b]["out"] + results[2 * b + 1]["out"]
    return out


# ---------------------------------------------------------------------------
# PJRT runner (device-resident inputs, reusable jitted executable)
# ---------------------------------------------------------------------------

class _Runner:
    def __init__(self, nc, n_cores=N_CORES):
        import jax
        import concourse.mybir as mybir
        from concourse import bass2jax
        from jax.sharding import Mesh, PartitionSpec, NamedSharding
        from jax.experimental.shard_map import shard_map

        self.jax = jax
        bass2jax.install_neuronx_cc_hook()
        partition_name = (
            nc.partition_id_tensor.name if nc.partition_id_tensor else None
        )
        in_names, out_names, out_avals, zero_outs = [], [], [], []
        for alloc in nc.m.functions[0].allocations:
            if not isinstance(alloc, mybir.MemoryLocationSet):
                continue
            name = alloc.memorylocations[0].name
            if alloc.kind == "ExternalInput":
                if name != partition_name:
                    in_names.append(name)
            elif alloc.kind == "ExternalOutput":
                out_names.append(name)
                shape = tuple(alloc.tensor_shape)
                dtype = mybir.dt.np(alloc.dtype)
                out_avals.append(jax.core.ShapedArray(shape, dtype))
                zero_outs.append(np.zeros(shape, dtype))
        self.in_names, self.out_names = in_names, out_names
        self.out_avals, self.zero_outs = out_avals, zero_outs
        self.n_cores = n_cores
        all_names = in_names + out_names
        if partition_name is not None:
            all_names = all_names + [partition_name]

        def _bdy(*args):
            operands = list(args)
            if partition_name is not None:
                operands.append(bass2jax.partition_id_tensor())
            outs = bass2jax._bass_exec_p.bind(
                *operands,
                out_avals=tuple(out_avals),
                in_names=tuple(all_names),
                out_names=tuple(out_names),
                lowering_input_output_aliases=(),
                sim_require_finite=True,
                sim_require_nnan=True,
                nc=nc,
            )
            return tuple(outs)

        devices = jax.devices()[:n_cores]
        mesh = Mesh(np.asarray(devices), ("core",))
        n_args = len(in_names) + len(out_names)
        self.fn = jax.jit(
            shard_map(
                _bdy, mesh=mesh,
                in_specs=(PartitionSpec("core"),) * n_args,
                out_specs=(PartitionSpec("core"),) * len(out_names),
                check_rep=False,
            ),
            keep_unused=True,
        )
        self.sharding = NamedSharding(mesh, PartitionSpec("core"))

    def put_inputs(self, in_maps):
        concat = [
            np.concatenate([np.asarray(m[name]) for m in in_maps], axis=0)
            for name in self.in_names
        ]
        concat += [
            np.zeros((self.n_cores * z.shape[0], *z.shape[1:]), z.dtype)
            for z in self.zero_outs
        ]
        self.args = [self.jax.device_put(a, self.sharding) for a in concat]
        self.jax.block_until_ready(self.args)

    def run(self):
        outs = self.fn(*self.args)
        self.jax.block_until_ready(outs)
        return [
            {
                name: np.asarray(outs[i]).reshape(
                    self.n_cores, *self.out_avals[i].shape)[c]
                for i, name in enumerate(self.out_names)
            }
            for c in range(self.n_cores)
        ]

    def time_ns(self, iters=20, warmup=2):
        import time
        for _ in range(warmup):
            self.jax.block_until_ready(self.fn(*self.args))
        t0 = time.perf_counter()
        outs = None
        for _ in range(iters):
            outs = self.fn(*self.args)
        self.jax.block_until_ready(outs)
        t1 = time.perf_counter()
        return (t1 - t0) / iters * 1e9



@functools.lru_cache(maxsize=None)
def _get_runner(rep=1, la=5, sbufs=2, ybufs=2, pbufs=2, ptbufs=8, ablate="full",
                norm="sbuf", fp8a=False, xbufs=1, vbufs=1):
    return _Runner(_get_nc(rep, la, sbufs, ybufs, pbufs, ptbufs, ablate, norm,
                           fp8a, xbufs, vbufs))


def kernel(x, w_qkv, w_proj):
    x = np.asarray(x, dtype=np.float32)
    w_qkv = np.asarray(w_qkv, dtype=np.float32)
    w_proj = np.asarray(w_proj, dtype=np.float32)
    runner = _get_runner()
    runner.put_inputs(make_in_maps(x, w_qkv, w_proj))
    return combine(runner.run())


# revision 44
# speedup vs baseline: 1.0453x; 1.0028x over previous
"""Causal self-attention v7 (B=4, T=2048, C=1024, H=16) on 8 trn2 NeuronCores.

Sharding: core = (batch b, head-group hg), b = core//2, hg = core%2 (Megatron
column-parallel qkv / row-parallel proj); host sums the two partial outputs.

Changes vs the 468us v2 baseline (measured ~283-289us):
  - qb-major schedule: proj tasks (a=QK, b=V, d=out-proj) woven INTO the
    attention group stream so the PE never idles while ACT (exp) works;
    d(qb-1) runs during C(qb).
  - normalization: DVE copy of the PSUM rowsum row (recip straight off PSUM
    is broken on HW - sim diverges), DVE reciprocal, GPSIMD
    partition_broadcast (idle engine) for the [64,512] broadcast, single DVE
    mul per head.  No more PE broadcast matmuls.
  - S/pt tiles are [128, 2, 512] so straddle groups get ONE exp over a
    strided AP instead of two (saves 352 ACT cycles per instruction).
  - proj PSUM tiles are one bank each ([128,512]) in their own pool, so an
    in-flight a/b/d task no longer blocks the S-tile pipeline.
  - PSUM budget: s 2x2 banks + y 2x1 + proj 2x1 = 8 banks.
  - yt stored per-pair (finer dep granularity for the tail d-tasks); v ones
    columns memset only (was a 7us whole-tile memset blocking DVE each rep).
  - batched DMAs (rearranged whole-tensor transfers, one out-DMA per
    t-tile): each dma_start costs ~600ns of HWDGE issue time.
  - fp8a (DoubleRow QK-projection) exists but is OFF: e4m3 q/k noise puts
    max-rel err at 3.5e-2 > 2e-2 tolerance.
  - lookahead la=5 with ptbufs=8 (exp/mask run ~5 groups ahead of the AV
    matmuls): rep3-trace sweep la=3/4/5/6 -> 844.8/833.8/827.6/832.6us.
  - DMA order wq, xt(tb0), wk: the first a-task's q-half matmuls start as
    soon as wq+xt0 land; wk rides behind (rep3 trace 827.6 -> 818.6us).
"""

import functools

import numpy as np

B, T, C, H = 4, 2048, 1024, 16
HD = C // H  # 64
N_CORES = 8
HG = 2  # head groups
NH = H // HG  # heads per core = 8
NP = NH // 2  # head pairs per core = 4
TT = T // 128  # 16 t-tiles
TB = T // 512  # 4 t-blocks
CK = C // 128  # 8 c-chunks


def _build(rep=1, la=5, sbufs=2, ybufs=2, pbufs=2, ptbufs=8, ablate="full",
           norm="sbuf", fp8a=False, xbufs=1, vbufs=1):
    import concourse.bass as bass
    import concourse.mybir as mybir
    import concourse.tile as tile
    from concourse import bacc

    f32 = mybir.dt.float32
    bf16 = mybir.dt.bfloat16
    f8 = mybir.dt.float8e4

    nc = bacc.Bacc("TRN2", target_bir_lowering=False, debug=False)

    xt_d = nc.dram_tensor("xt", [C, T], bf16, kind="ExternalInput")
    if fp8a:
        # fp8 copies for the DoubleRow QK projection; wq8/wk8 are already
        # interleaved host-side as [ki=128, kpair=4, j=2, m=512] and carry a
        # x32 pre-scale (compensated in the exp scale) to clear the e4m3
        # subnormal range.
        xt8_d = nc.dram_tensor("xt8", [C, T], f8, kind="ExternalInput")
        wq8_d = nc.dram_tensor("wq8", [128, 4096], f8, kind="ExternalInput")
        wk8_d = nc.dram_tensor("wk8", [128, 4096], f8, kind="ExternalInput")
    else:
        wq_d = nc.dram_tensor("wq", [C, 512], bf16, kind="ExternalInput")
        wk_d = nc.dram_tensor("wk", [C, 512], bf16, kind="ExternalInput")
    wv_d = nc.dram_tensor("wv", [C, 512], bf16, kind="ExternalInput")
    wp_d = nc.dram_tensor("wp", [512, C], bf16, kind="ExternalInput")
    tri_d = nc.dram_tensor("tri", [128, 128], bf16, kind="ExternalInput")
    out_d = nc.dram_tensor("out", [T, C], f32, kind="ExternalOutput")

    exp_scale = 0.125 / 1024.0 if fp8a else 0.125

    do_attn = ablate in ("full", "noav")
    do_av = ablate in ("full",)
    do_d = ablate in ("full", "noattn")

    with tile.TileContext(nc) as tc:
        with tc.tile_pool(name="persist", bufs=1) as persist:
            qt_sb = persist.tile([128, NP, T], bf16, tag="qt")
            kt_sb = persist.tile([128, NP, T], bf16, tag="kt")

            def body():
                # strictly nested (LIFO) pool lifetimes
                vp_cm = tc.tile_pool(name="vp", bufs=vbufs)
                xtp_cm = tc.tile_pool(name="xtp", bufs=xbufs)
                wqk_cm = tc.tile_pool(name="wqk", bufs=1)
                vp = vp_cm.__enter__()
                xtp = xtp_cm.__enter__()
                wqk = wqk_cm.__enter__()

                v_sb = vp.tile([128, TT, NH, HD + 1], bf16, tag="v")
                # ones columns of V' only (V-proj copies fill 0:HD)
                nc.vector.memset(v_sb[:, :, :, HD:HD + 1], 1.0)

                if fp8a:
                    wq8_sb = wqk.tile([128, 4, 2, 512], f8, tag="wq8")
                    wk8_sb = wqk.tile([128, 4, 2, 512], f8, tag="wk8")
                    xt8_sb = xtp.tile([128, CK, T], f8, tag="xt8")
                else:
                    wq_sb = wqk.tile([128, CK, 512], bf16, tag="wq")
                    wk_sb = wqk.tile([128, CK, 512], bf16, tag="wk")
                wv_sb = wqk.tile([128, CK, 512], bf16, tag="wv")
                wp_sb = wqk.tile([128, NP, C], bf16, tag="wp")
                tri_sb = wqk.tile([128, 128], bf16, tag="tri")
                xt_sb = xtp.tile([128, CK, T], bf16, tag="xt")
                # exp-table preload on the idle ACT engine during the DMA head
                warm = wqk.tile([128, 32], bf16, tag="warm")
                nc.vector.memset(warm[:], 1.0)
                nc.scalar.activation(
                    warm[0:1, 16:32], warm[0:1, 0:16],
                    mybir.ActivationFunctionType.Exp, scale=exp_scale,
                )
                # chunked DMAs in consumption order; tri first (first-unit
                # masks); QK operands for tb=0 first so phase A starts early.
                nc.sync.dma_start(tri_sb[:], tri_d[:, :])
                if fp8a:
                    nc.sync.dma_start(wq8_sb[:], wq8_d.rearrange(
                        "p (a b n) -> p a b n", a=4, b=2))
                    nc.sync.dma_start(wk8_sb[:], wk8_d.rearrange(
                        "p (a b n) -> p a b n", a=4, b=2))
                    for tb in range(TB):
                        tsl = slice(tb * 512, (tb + 1) * 512)
                        for k in range(CK):
                            ksl = slice(k * 128, (k + 1) * 128)
                            nc.sync.dma_start(
                                xt8_sb[:, k, tsl], xt8_d[ksl, tsl])
                    for k in range(CK):
                        ksl = slice(k * 128, (k + 1) * 128)
                        nc.sync.dma_start(xt_sb[:, k, 0:512], xt_d[ksl, 0:512])
                        nc.sync.dma_start(wv_sb[:, k, :], wv_d[ksl, :])
                else:
                    nc.sync.dma_start(
                        wq_sb[:], wq_d.rearrange("(a p) n -> p a n", p=128))
                    nc.sync.dma_start(
                        xt_sb[:, :, 0:512],
                        xt_d[:, 0:512].rearrange("(a p) t -> p a t", p=128))
                    nc.sync.dma_start(
                        wk_sb[:], wk_d.rearrange("(a p) n -> p a n", p=128))
                    nc.sync.dma_start(
                        wv_sb[:], wv_d.rearrange("(a p) n -> p a n", p=128))
                for tb in range(1, TB):
                    tsl = slice(tb * 512, (tb + 1) * 512)
                    nc.sync.dma_start(
                        xt_sb[:, :, tsl],
                        xt_d[:, tsl].rearrange("(a p) t -> p a t", p=128))
                nc.sync.dma_start(wp_sb[:], wp_d.rearrange("(a p) n -> p a n", p=128))

                with (
                    tc.tile_pool(name="persist2", bufs=1) as persist2,
                    tc.tile_pool(name="ptp", bufs=ptbufs) as ptp,
                    tc.tile_pool(name="recp", bufs=2) as recp,
                    tc.tile_pool(name="bcp", bufs=2) as bcp,
                    tc.tile_pool(name="outp", bufs=4) as outp,
                ):
                    yt_sb = [
                        persist2.tile([128, T], bf16, tag=f"yt{p}",
                                      name=f"yt{p}")
                        for p in range(NP)
                    ]
                    if ablate in ("noattn",):
                        for p in range(NP):
                            nc.vector.memset(yt_sb[p][:], 0.001)
                    with (
                        tc.tile_pool(name="pss", bufs=sbufs, space="PSUM") as pss,
                        tc.tile_pool(name="psy", bufs=ybufs, space="PSUM") as psy,
                        tc.tile_pool(name="psp", bufs=pbufs, space="PSUM") as psp,
                    ):
                        # ---- task list: qb-major, proj tasks woven in
                        tasks = []
                        if do_attn:
                            for pp in range(NP):
                                tasks.append(("a", pp, 0, 0, 0))
                            for tt in range(4):
                                tasks.append(("b", tt, 0, 0, 0))
                            for qb in range(TB):
                                inter = []
                                if qb < TB - 1:
                                    for pp in range(NP):
                                        inter.append(("a", pp, qb + 1, 0, 0))
                                if qb == 0:
                                    for tt in range(4, 8):
                                        inter.append(("b", tt, 0, 0, 0))
                                if do_d and qb > 0:
                                    for tt in range(4 * (qb - 1), 4 * qb):
                                        inter.append(("d", tt, 0, 0, 0))
                                ng = 4 * (qb + 1)
                                glist = [
                                    ("g", qb, p, g, ng)
                                    for p in range(NP)
                                    for g in range(ng)
                                ]
                                if qb >= 2:
                                    # JIT: this qb's own V tiles, inserted
                                    # strictly BEFORE their first consumer
                                    # group (p0, g=tt) with margin
                                    for j, tt in enumerate(
                                            range(4 * qb, 4 * qb + 4)):
                                        glist.insert(
                                            4 * (qb - 1) + 2 * j,
                                            ("b", tt, 0, 0, 0))
                                if inter:
                                    step = max(1, len(glist) // len(inter))
                                    woven, ii = [], 0
                                    for j, t in enumerate(glist):
                                        woven.append(t)
                                        if j % step == step - 1 and ii < len(inter):
                                            woven.append(inter[ii])
                                            ii += 1
                                    woven += inter[ii:]
                                    glist = woven
                                tasks += glist
                            if do_d:
                                for tt in range(TT - 4, TT):
                                    tasks.append(("d", tt, 0, 0, 0))
                        elif do_d:
                            for tt in range(TT):
                                tasks.append(("d", tt, 0, 0, 0))

                        pt_store = {}
                        s_store = {}
                        y_store = {}

                        def emit_front(idx):
                            kind, qb, p, g, ng = tasks[idx]
                            if kind != "g":
                                return
                            r = g - 4 * qb  # >=0: diagonal-straddling chunk
                            lo = 128 * r if r > 0 else 0
                            ksl = slice(g * 128, (g + 1) * 128)
                            s = pss.tile([128, 2, 512], f32, tag="s",
                                         name=f"s_{idx}")
                            for hf in range(2):
                                nc.tensor.matmul(
                                    s[:, hf, lo:512],
                                    kt_sb[64 * hf: 64 * (hf + 1), p, ksl],
                                    qt_sb[64 * hf: 64 * (hf + 1), p,
                                          qb * 512 + lo: (qb + 1) * 512],
                                    start=True, stop=True,
                                )
                            pt = ptp.tile([128, 2, 512], bf16, tag="pt",
                                          name=f"pt_{idx}")
                            # one exp per group (strided AP when straddling)
                            nc.scalar.activation(
                                pt[:, :, lo:512], s[:, :, lo:512],
                                mybir.ActivationFunctionType.Exp,
                                scale=exp_scale,
                            )
                            if r >= 0:
                                # triangle mask on the diagonal 128-wide strip
                                for hf in range(2):
                                    nc.vector.tensor_mul(
                                        pt[:, hf, lo:lo + 128],
                                        pt[:, hf, lo:lo + 128], tri_sb[:])
                            pt_store[idx] = pt

                        def emit_back(idx):
                            kind, qb, p, g, ng = tasks[idx]
                            if kind == "a":
                                pp, tb = qb, p
                                tsl = slice(tb * 512, (tb + 1) * 512)
                                psl = slice(pp * 128, (pp + 1) * 128)
                                psq = psp.tile([128, 512], f32, tag="o",
                                               name=f"psq{pp}_{tb}")
                                if fp8a:
                                    for kp in range(4):
                                        nc.tensor.matmul(
                                            psq[:], wq8_sb[:, kp, :, psl],
                                            xt8_sb[:, 2 * kp:2 * kp + 2, tsl],
                                            start=(kp == 0), stop=(kp == 3),
                                            perf_mode=(
                                                mybir.MatmulPerfMode.DoubleRow),
                                        )
                                else:
                                    for k in range(CK):
                                        nc.tensor.matmul(
                                            psq[:], wq_sb[:, k, psl],
                                            xt_sb[:, k, tsl],
                                            start=(k == 0), stop=(k == CK - 1),
                                        )
                                nc.vector.tensor_copy(qt_sb[:, pp, tsl], psq[:])
                                psk = psp.tile([128, 512], f32, tag="o",
                                               name=f"psk{pp}_{tb}")
                                if fp8a:
                                    for kp in range(4):
                                        nc.tensor.matmul(
                                            psk[:], wk8_sb[:, kp, :, psl],
                                            xt8_sb[:, 2 * kp:2 * kp + 2, tsl],
                                            start=(kp == 0), stop=(kp == 3),
                                            perf_mode=(
                                                mybir.MatmulPerfMode.DoubleRow),
                                        )
                                else:
                                    for k in range(CK):
                                        nc.tensor.matmul(
                                            psk[:], wk_sb[:, k, psl],
                                            xt_sb[:, k, tsl],
                                            start=(k == 0), stop=(k == CK - 1),
                                        )
                                nc.vector.tensor_copy(kt_sb[:, pp, tsl], psk[:])
                                return
                            if kind == "b":
                                tt = qb
                                psv = psp.tile([128, 512], f32, tag="o",
                                               name=f"psv{tt}")
                                for k in range(CK):
                                    nc.tensor.matmul(
                                        psv[:],
                                        xt_sb[:, k, tt * 128: (tt + 1) * 128],
                                        wv_sb[:, k, :],
                                        start=(k == 0), stop=(k == CK - 1),
                                    )
                                nc.vector.tensor_copy(
                                    v_sb[:, tt, :, 0:HD],
                                    psv[:].rearrange("p (h e) -> p h e", e=HD),
                                )
                                return
                            if kind == "d":
                                tt = qb
                                tsl = slice(tt * 128, (tt + 1) * 128)
                                ot = outp.tile([128, 1024], f32, tag="ot")
                                for nb in range(2):
                                    po = psp.tile([128, 512], f32, tag="o",
                                                  name=f"po{tt}_{nb}")
                                    for pp2 in range(NP):
                                        nc.tensor.matmul(
                                            po[:],
                                            yt_sb[pp2][:, tsl],
                                            wp_sb[:, pp2,
                                                  nb * 512: (nb + 1) * 512],
                                            start=(pp2 == 0),
                                            stop=(pp2 == NP - 1),
                                        )
                                    nc.vector.tensor_copy(
                                        ot[:, nb * 512: (nb + 1) * 512], po[:])
                                nc.sync.dma_start(out_d[tsl, :], ot[:])
                                return
                            # kind == "g"
                            r = g - 4 * qb
                            lo = 128 * r if r > 0 else 0
                            pt = pt_store.pop(idx)
                            if (qb, p) not in y_store:
                                ya = psy.tile([65, 512], f32, tag="y",
                                              name=f"ya_{qb}_{p}")
                                yb = psy.tile([65, 512], f32, tag="y",
                                              name=f"yb_{qb}_{p}")
                                y_store[(qb, p)] = (ya, yb)
                            ya, yb = y_store[(qb, p)]
                            if do_av:
                                for hf, yy in ((0, ya), (1, yb)):
                                    nc.tensor.matmul(
                                        yy[:, lo:512],
                                        v_sb[:, g, 2 * p + hf, :],
                                        pt[:, hf, lo:512],
                                        start=(g == 0), stop=(g == ng - 1),
                                        skip_group_check=True,
                                    )
                            if g != ng - 1:
                                return
                            if not do_av:
                                nc.vector.memset(ya[:], 1.0)
                                nc.vector.memset(yb[:], 1.0)
                            # normalize: yt = y[0:64] * (1 / rowsum)
                            qsl = slice(qb * 512, (qb + 1) * 512)
                            for hi, yy in ((0, ya), (1, yb)):
                                rec = recp.tile([1, 512], f32, tag="rec",
                                                name=f"rec_{qb}_{p}_{hi}")
                                if norm == "gps":
                                    nc.vector.reciprocal_approx_fast(
                                        rec[0:1, :], yy[64:65, :])
                                else:  # "sbuf": stage rowsum in SBUF first
                                    rs = recp.tile([1, 512], f32, tag="rs",
                                                   name=f"rs_{qb}_{p}_{hi}")
                                    nc.vector.tensor_copy(
                                        rs[0:1, :], yy[64:65, :])
                                    nc.vector.reciprocal_approx_fast(
                                        rec[0:1, :], rs[0:1, :])
                                bc = bcp.tile([64, 512], f32, tag="bc",
                                              name=f"bc_{qb}_{p}_{hi}")
                                nc.gpsimd.partition_broadcast(
                                    bc[:], rec[0:1, :], channels=64)
                                nc.vector.tensor_mul(
                                    yt_sb[p][hi * 64: (hi + 1) * 64, qsl],
                                    yy[0:64, :], bc[:],
                                )
                            del y_store[(qb, p)]

                        n = len(tasks)
                        for j in range(min(la, n)):
                            emit_front(j)
                        for i in range(n):
                            if i + la < n:
                                emit_front(i + la)
                            emit_back(i)

                wqk_cm.__exit__(None, None, None)
                xtp_cm.__exit__(None, None, None)
                vp_cm.__exit__(None, None, None)

            if rep == 1:
                body()
            else:
                with tc.For_i(0, rep, 1):
                    body()

    nc.compile()
    return nc


@functools.lru_cache(maxsize=None)
def _get_nc(rep=1, la=5, sbufs=2, ybufs=2, pbufs=2, ptbufs=8, ablate="full",
            norm="sbuf", fp8a=False, xbufs=1, vbufs=1):
    return _build(rep, la, sbufs, ybufs, pbufs, ptbufs, ablate, norm, fp8a,
                  xbufs, vbufs)


FP8A = False  # must match the _build/_get_runner default


def make_in_maps(x, w_qkv, w_proj):
    import ml_dtypes
    bf16 = ml_dtypes.bfloat16
    j = np.arange(128)[None, :]
    i = np.arange(128)[:, None]
    tri = (j >= i).astype(bf16)

    in_maps = []
    for core in range(N_CORES):
        b, hg = divmod(core, HG)
        sl = slice(hg * 512, (hg + 1) * 512)
        xtb = np.ascontiguousarray(x[b].T)
        wqt = np.ascontiguousarray(w_qkv[sl].T)
        wkt = np.ascontiguousarray(w_qkv[C:2 * C][sl].T)
        m = {
            "xt": xtb.astype(bf16),
            "wq": wqt.astype(bf16),
            "wk": wkt.astype(bf16),
            "wv": np.ascontiguousarray(w_qkv[2 * C:3 * C][sl].T).astype(bf16),
            "wp": np.ascontiguousarray(w_proj[:, sl].T).astype(bf16),
            "tri": tri,
        }
        if FP8A:
            f8 = ml_dtypes.float8_e4m3

            def dr8(wt):
                # [C,512] -> DoubleRow-interleaved [128, kp*j*m] fp8, x32
                return np.ascontiguousarray(
                    (wt * 32.0).reshape(4, 2, 128, 512).transpose(2, 0, 1, 3)
                    .reshape(128, 4096)).astype(f8)

            m["xt8"] = xtb.astype(f8)
            m["wq8"] = dr8(wqt)
            m["wk8"] = dr8(wkt)
        in_maps.append(m)
    return in_maps


def combine(results):
    out = np.empty((B, T, C), dtype=np.float32)
    for b in range(B):
        out[b] = results[2 * b]["out"] + results[2 * b + 1]["out"]
    return out


# ---------------------------------------------------------------------------
# PJRT runner (device-resident inputs, reusable jitted executable)
# ---------------------------------------------------------------------------

class _Runner:
    def __init__(self, nc, n_cores=N_CORES):
        import jax
        import concourse.mybir as mybir
        from concourse import bass2jax
        from jax.sharding import Mesh, PartitionSpec, NamedSharding
        from jax.experimental.shard_map import shard_map

        self.jax = jax
        bass2jax.install_neuronx_cc_hook()
        partition_name = (
            nc.partition_id_tensor.name if nc.partition_id_tensor else None
        )
        in_names, out_names, out_avals, zero_outs = [], [], [], []
        for alloc in nc.m.functions[0].allocations:
            if not isinstance(alloc, mybir.MemoryLocationSet):
                continue
            name = alloc.memorylocations[0].name
            if alloc.kind == "ExternalInput":
                if name != partition_name:
                    in_names.append(name)
            elif alloc.kind == "ExternalOutput":
                out_names.append(name)
                shape = tuple(alloc.tensor_shape)
                dtype = mybir.dt.np(alloc.dtype)
                out_avals.append(jax.core.ShapedArray(shape, dtype))
                zero_outs.append(np.zeros(shape, dtype))
        self.in_names, self.out_names = in_names, out_names
        self.out_avals, self.zero_outs = out_avals, zero_outs
        self.n_cores = n_cores
        all_names = in_names + out_names
        if partition_name is not None:
            all_names = all_names + [partition_name]

        def _bdy(*args):
            operands = list(args)
            if partition_name is not None:
                operands.append(bass2jax.partition_id_tensor())
            outs = bass2jax._bass_exec_p.bind(
                *operands,
                out_avals=tuple(out_avals),
                in_names=tuple(all_names),
                out_names=tuple(out_names),
                lowering_input_output_aliases=(),
                sim_require_finite=True,
                sim_require_nnan=True,
                nc=nc,
            )
            return tuple(outs)

        devices = jax.devices()[:n_cores]
        mesh = Mesh(np.asarray(devices), ("core",))
        n_args = len(in_names) + len(out_names)
        self.fn = jax.jit(
            shard_map(
                _bdy, mesh=mesh,
                in_specs=(PartitionSpec("core"),) * n_args,
                out_specs=(PartitionSpec("core"),) * len(out_names),
                check_rep=False,
            ),
            keep_unused=True,
        )
        self.sharding = NamedSharding(mesh, PartitionSpec("core"))

    def put_inputs(self, in_maps):
        concat = [
            np.concatenate([np.asarray(m[name]) for m in in_maps], axis=0)
            for name in self.in_names
        ]
        concat += [
            np.zeros((self.n_cores * z.shape[0], *z.shape[1:]), z.dtype)
            for z in self.zero_outs
        ]
        self.args = [self.jax.device_put(a, self.sharding) for a in concat]
        self.jax.block_until_ready(self.args)

    def run(self):
        outs = self.fn(*self.args)
        self.jax.block_until_ready(outs)
        return [
            {
                name: np.asarray(outs[i]).reshape(
                    self.n_cores, *self.out_avals[i].shape)[c]
                for i, name in enumerate(self.out_names)
            }
            for c in range(self.n_cores)
        ]

    def time_ns(self, iters=20, warmup=2):
        import time
        for _ in range(warmup):
            self.jax.block_until_ready(self.fn(*self.args))
        t0 = time.perf_counter()
        outs = None
        for _ in range(iters):
            outs = self.fn(*self.args)
        self.jax.block_until_ready(outs)
        t1 = time.perf_counter()
        return (t1 - t0) / iters * 1e9



@functools.lru_cache(maxsize=None)
def _get_runner(rep=1, la=5, sbufs=2, ybufs=2, pbufs=2, ptbufs=8, ablate="full",
                norm="sbuf", fp8a=False, xbufs=1, vbufs=1):
    return _Runner(_get_nc(rep, la, sbufs, ybufs, pbufs, ptbufs, ablate, norm,
                           fp8a, xbufs, vbufs))


def kernel(x, w_qkv, w_proj):
    x = np.asarray(x, dtype=np.float32)
    w_qkv = np.asarray(w_qkv, dtype=np.float32)
    w_proj = np.asarray(w_proj, dtype=np.float32)
    runner = _get_runner()
    runner.put_inputs(make_in_maps(x, w_qkv, w_proj))
    return combine(runner.run())


# revision 45
# speedup vs baseline: 1.0834x; 1.0364x over previous
"""Causal self-attention v7 (B=4, T=2048, C=1024, H=16) on 8 trn2 NeuronCores.

Sharding: core = (batch b, head-group hg), b = core//2, hg = core%2 (Megatron
column-parallel qkv / row-parallel proj); host sums the two partial outputs.

Changes vs the 468us v2 baseline (measured ~283-289us):
  - qb-major schedule: proj tasks (a=QK, b=V, d=out-proj) woven INTO the
    attention group stream so the PE never idles while ACT (exp) works;
    d(qb-1) runs during C(qb).
  - normalization: DVE copy of the PSUM rowsum row (recip straight off PSUM
    is broken on HW - sim diverges), DVE reciprocal, GPSIMD
    partition_broadcast (idle engine) for the [64,512] broadcast, single DVE
    mul per head.  No more PE broadcast matmuls.
  - S/pt tiles are [128, 2, 512] so straddle groups get ONE exp over a
    strided AP instead of two (saves 352 ACT cycles per instruction).
  - proj PSUM tiles are one bank each ([128,512]) in their own pool, so an
    in-flight a/b/d task no longer blocks the S-tile pipeline.
  - PSUM budget: s 2x2 banks + y 2x1 + proj 2x1 = 8 banks.
  - yt stored per-pair (finer dep granularity for the tail d-tasks); v ones
    columns memset only (was a 7us whole-tile memset blocking DVE each rep).
  - batched DMAs (rearranged whole-tensor transfers, one out-DMA per
    t-tile): each dma_start costs ~600ns of HWDGE issue time.
  - fp8a (DoubleRow QK-projection) exists but is OFF: e4m3 q/k noise puts
    max-rel err at 3.5e-2 > 2e-2 tolerance.
  - lookahead la=5 with ptbufs=8 (exp/mask run ~5 groups ahead of the AV
    matmuls): rep3-trace sweep la=3/4/5/6 -> 844.8/833.8/827.6/832.6us.
  - DMA order wq, xt(tb0), wk: the first a-task's q-half matmuls start as
    soon as wq+xt0 land; wk rides behind (rep3 trace 827.6 -> 818.6us).
"""

import functools

import numpy as np

B, T, C, H = 4, 2048, 1024, 16
HD = C // H  # 64
N_CORES = 8
HG = 2  # head groups
NH = H // HG  # heads per core = 8
NP = NH // 2  # head pairs per core = 4
TT = T // 128  # 16 t-tiles
TB = T // 512  # 4 t-blocks
CK = C // 128  # 8 c-chunks


def _build(rep=1, la=5, sbufs=2, ybufs=2, pbufs=2, ptbufs=8, ablate="full",
           norm="sbuf", fp8a=False, xbufs=1, vbufs=1):
    import concourse.bass as bass
    import concourse.mybir as mybir
    import concourse.tile as tile
    from concourse import bacc

    f32 = mybir.dt.float32
    bf16 = mybir.dt.bfloat16
    f8 = mybir.dt.float8e4

    nc = bacc.Bacc("TRN2", target_bir_lowering=False, debug=False)

    xt_d = nc.dram_tensor("xt", [C, T], bf16, kind="ExternalInput")
    if fp8a:
        # fp8 copies for the DoubleRow QK projection; wq8/wk8 are already
        # interleaved host-side as [ki=128, kpair=4, j=2, m=512] and carry a
        # x32 pre-scale (compensated in the exp scale) to clear the e4m3
        # subnormal range.
        xt8_d = nc.dram_tensor("xt8", [C, T], f8, kind="ExternalInput")
        wq8_d = nc.dram_tensor("wq8", [128, 4096], f8, kind="ExternalInput")
        wk8_d = nc.dram_tensor("wk8", [128, 4096], f8, kind="ExternalInput")
    else:
        wq_d = nc.dram_tensor("wq", [C, 512], bf16, kind="ExternalInput")
        wk_d = nc.dram_tensor("wk", [C, 512], bf16, kind="ExternalInput")
    wv_d = nc.dram_tensor("wv", [C, 512], bf16, kind="ExternalInput")
    wp_d = nc.dram_tensor("wp", [512, C], bf16, kind="ExternalInput")
    tri_d = nc.dram_tensor("tri", [128, 128], bf16, kind="ExternalInput")
    out_d = nc.dram_tensor("out", [T, C], f32, kind="ExternalOutput")

    exp_scale = 0.125 / 1024.0 if fp8a else 0.125

    do_attn = ablate in ("full", "noav")
    do_av = ablate in ("full",)
    do_d = ablate in ("full", "noattn")

    with tile.TileContext(nc) as tc:
        with tc.tile_pool(name="persist", bufs=1) as persist:
            qt_sb = persist.tile([128, NP, T], bf16, tag="qt")
            kt_sb = persist.tile([128, NP, T], bf16, tag="kt")

            def body():
                # strictly nested (LIFO) pool lifetimes
                vp_cm = tc.tile_pool(name="vp", bufs=vbufs)
                xtp_cm = tc.tile_pool(name="xtp", bufs=xbufs)
                wqk_cm = tc.tile_pool(name="wqk", bufs=1)
                vp = vp_cm.__enter__()
                xtp = xtp_cm.__enter__()
                wqk = wqk_cm.__enter__()

                v_sb = vp.tile([128, TT, NH, HD + 1], bf16, tag="v")
                # ones columns of V' only (V-proj copies fill 0:HD)
                nc.vector.memset(v_sb[:, :, :, HD:HD + 1], 1.0)

                if fp8a:
                    wq8_sb = wqk.tile([128, 4, 2, 512], f8, tag="wq8")
                    wk8_sb = wqk.tile([128, 4, 2, 512], f8, tag="wk8")
                    xt8_sb = xtp.tile([128, CK, T], f8, tag="xt8")
                else:
                    wq_sb = wqk.tile([128, CK, 512], bf16, tag="wq")
                    wk_sb = wqk.tile([128, CK, 512], bf16, tag="wk")
                wv_sb = wqk.tile([128, CK, 512], bf16, tag="wv")
                wp_sb = wqk.tile([128, NP, C], bf16, tag="wp")
                tri_sb = wqk.tile([128, 128], bf16, tag="tri")
                xt_sb = xtp.tile([128, CK, T], bf16, tag="xt")
                # exp-table preload on the idle ACT engine during the DMA head
                warm = wqk.tile([128, 32], bf16, tag="warm")
                nc.vector.memset(warm[:], 1.0)
                nc.scalar.activation(
                    warm[0:1, 16:32], warm[0:1, 0:16],
                    mybir.ActivationFunctionType.Exp, scale=exp_scale,
                )
                # chunked DMAs in consumption order; tri first (first-unit
                # masks); QK operands for tb=0 first so phase A starts early.
                nc.sync.dma_start(tri_sb[:], tri_d[:, :])
                if fp8a:
                    nc.sync.dma_start(wq8_sb[:], wq8_d.rearrange(
                        "p (a b n) -> p a b n", a=4, b=2))
                    nc.sync.dma_start(wk8_sb[:], wk8_d.rearrange(
                        "p (a b n) -> p a b n", a=4, b=2))
                    for tb in range(TB):
                        tsl = slice(tb * 512, (tb + 1) * 512)
                        for k in range(CK):
                            ksl = slice(k * 128, (k + 1) * 128)
                            nc.sync.dma_start(
                                xt8_sb[:, k, tsl], xt8_d[ksl, tsl])
                    for k in range(CK):
                        ksl = slice(k * 128, (k + 1) * 128)
                        nc.sync.dma_start(xt_sb[:, k, 0:512], xt_d[ksl, 0:512])
                        nc.sync.dma_start(wv_sb[:, k, :], wv_d[ksl, :])
                else:
                    nc.sync.dma_start(
                        wq_sb[:], wq_d.rearrange("(a p) n -> p a n", p=128))
                    nc.sync.dma_start(
                        xt_sb[:, :, 0:512],
                        xt_d[:, 0:512].rearrange("(a p) t -> p a t", p=128))
                    nc.sync.dma_start(
                        wk_sb[:], wk_d.rearrange("(a p) n -> p a n", p=128))
                    nc.sync.dma_start(
                        wv_sb[:], wv_d.rearrange("(a p) n -> p a n", p=128))
                for tb in range(1, TB):
                    tsl = slice(tb * 512, (tb + 1) * 512)
                    nc.sync.dma_start(
                        xt_sb[:, :, tsl],
                        xt_d[:, tsl].rearrange("(a p) t -> p a t", p=128))
                nc.sync.dma_start(wp_sb[:], wp_d.rearrange("(a p) n -> p a n", p=128))

                with (
                    tc.tile_pool(name="persist2", bufs=1) as persist2,
                    tc.tile_pool(name="ptp", bufs=ptbufs) as ptp,
                    tc.tile_pool(name="recp", bufs=2) as recp,
                    tc.tile_pool(name="bcp", bufs=2) as bcp,
                    tc.tile_pool(name="outp", bufs=4) as outp,
                ):
                    yt_sb = [
                        persist2.tile([128, T], bf16, tag=f"yt{p}",
                                      name=f"yt{p}")
                        for p in range(NP)
                    ]
                    if ablate in ("noattn",):
                        for p in range(NP):
                            nc.vector.memset(yt_sb[p][:], 0.001)
                    with (
                        tc.tile_pool(name="pss", bufs=sbufs, space="PSUM") as pss,
                        tc.tile_pool(name="psy", bufs=ybufs, space="PSUM") as psy,
                        tc.tile_pool(name="psp", bufs=pbufs, space="PSUM") as psp,
                    ):
                        # ---- task list: qb-major, proj tasks woven in
                        tasks = []
                        if do_attn:
                            for pp in range(NP):
                                tasks.append(("a", pp, 0, 0, 0))
                            for tt in range(4):
                                tasks.append(("b", tt, 0, 0, 0))
                            for qb in range(TB):
                                inter = []
                                if qb < TB - 1:
                                    for pp in range(NP):
                                        inter.append(("a", pp, qb + 1, 0, 0))
                                    for tt in range(4 * qb + 4, 4 * qb + 8):
                                        inter.append(("b", tt, 0, 0, 0))
                                if do_d and qb > 0:
                                    for tt in range(4 * (qb - 1), 4 * qb):
                                        inter.append(("d", tt, 0, 0, 0))
                                ng = 4 * (qb + 1)
                                glist = [
                                    ("g", qb, p, g, ng)
                                    for p in range(NP)
                                    for g in range(ng)
                                ]
                                if inter:
                                    step = max(1, len(glist) // len(inter))
                                    woven, ii = [], 0
                                    for j, t in enumerate(glist):
                                        woven.append(t)
                                        if j % step == step - 1 and ii < len(inter):
                                            woven.append(inter[ii])
                                            ii += 1
                                    woven += inter[ii:]
                                    glist = woven
                                tasks += glist
                            if do_d:
                                for tt in range(TT - 4, TT):
                                    tasks.append(("d", tt, 0, 0, 0))
                        elif do_d:
                            for tt in range(TT):
                                tasks.append(("d", tt, 0, 0, 0))

                        pt_store = {}
                        s_store = {}
                        y_store = {}

                        def emit_front(idx):
                            kind, qb, p, g, ng = tasks[idx]
                            if kind != "g":
                                return
                            r = g - 4 * qb  # >=0: diagonal-straddling chunk
                            lo = 128 * r if r > 0 else 0
                            ksl = slice(g * 128, (g + 1) * 128)
                            s = pss.tile([128, 2, 512], f32, tag="s",
                                         name=f"s_{idx}")
                            for hf in range(2):
                                nc.tensor.matmul(
                                    s[:, hf, lo:512],
                                    kt_sb[64 * hf: 64 * (hf + 1), p, ksl],
                                    qt_sb[64 * hf: 64 * (hf + 1), p,
                                          qb * 512 + lo: (qb + 1) * 512],
                                    start=True, stop=True,
                                )
                            pt = ptp.tile([128, 2, 512], bf16, tag="pt",
                                          name=f"pt_{idx}")
                            # one exp per group (strided AP when straddling)
                            nc.scalar.activation(
                                pt[:, :, lo:512], s[:, :, lo:512],
                                mybir.ActivationFunctionType.Exp,
                                scale=exp_scale,
                            )
                            if r >= 0:
                                # triangle mask on the diagonal 128-wide strip
                                for hf in range(2):
                                    nc.vector.tensor_mul(
                                        pt[:, hf, lo:lo + 128],
                                        pt[:, hf, lo:lo + 128], tri_sb[:])
                            pt_store[idx] = pt

                        def emit_back(idx):
                            kind, qb, p, g, ng = tasks[idx]
                            if kind == "a":
                                pp, tb = qb, p
                                tsl = slice(tb * 512, (tb + 1) * 512)
                                psl = slice(pp * 128, (pp + 1) * 128)
                                psq = psp.tile([128, 512], f32, tag="o",
                                               name=f"psq{pp}_{tb}")
                                if fp8a:
                                    for kp in range(4):
                                        nc.tensor.matmul(
                                            psq[:], wq8_sb[:, kp, :, psl],
                                            xt8_sb[:, 2 * kp:2 * kp + 2, tsl],
                                            start=(kp == 0), stop=(kp == 3),
                                            perf_mode=(
                                                mybir.MatmulPerfMode.DoubleRow),
                                        )
                                else:
                                    for k in range(CK):
                                        nc.tensor.matmul(
                                            psq[:], wq_sb[:, k, psl],
                                            xt_sb[:, k, tsl],
                                            start=(k == 0), stop=(k == CK - 1),
                                        )
                                nc.vector.tensor_copy(qt_sb[:, pp, tsl], psq[:])
                                psk = psp.tile([128, 512], f32, tag="o",
                                               name=f"psk{pp}_{tb}")
                                if fp8a:
                                    for kp in range(4):
                                        nc.tensor.matmul(
                                            psk[:], wk8_sb[:, kp, :, psl],
                                            xt8_sb[:, 2 * kp:2 * kp + 2, tsl],
                                            start=(kp == 0), stop=(kp == 3),
                                            perf_mode=(
                                                mybir.MatmulPerfMode.DoubleRow),
                                        )
                                else:
                                    for k in range(CK):
                                        nc.tensor.matmul(
                                            psk[:], wk_sb[:, k, psl],
                                            xt_sb[:, k, tsl],
                                            start=(k == 0), stop=(k == CK - 1),
                                        )
                                nc.vector.tensor_copy(kt_sb[:, pp, tsl], psk[:])
                                return
                            if kind == "b":
                                tt = qb
                                psv = psp.tile([128, 512], f32, tag="o",
                                               name=f"psv{tt}")
                                for k in range(CK):
                                    nc.tensor.matmul(
                                        psv[:],
                                        xt_sb[:, k, tt * 128: (tt + 1) * 128],
                                        wv_sb[:, k, :],
                                        start=(k == 0), stop=(k == CK - 1),
                                    )
                                nc.vector.tensor_copy(
                                    v_sb[:, tt, :, 0:HD],
                                    psv[:].rearrange("p (h e) -> p h e", e=HD),
                                )
                                return
                            if kind == "d":
                                tt = qb
                                tsl = slice(tt * 128, (tt + 1) * 128)
                                ot = outp.tile([128, 1024], f32, tag="ot")
                                for nb in range(2):
                                    po = psp.tile([128, 512], f32, tag="o",
                                                  name=f"po{tt}_{nb}")
                                    for pp2 in range(NP):
                                        nc.tensor.matmul(
                                            po[:],
                                            yt_sb[pp2][:, tsl],
                                            wp_sb[:, pp2,
                                                  nb * 512: (nb + 1) * 512],
                                            start=(pp2 == 0),
                                            stop=(pp2 == NP - 1),
                                        )
                                    nc.vector.tensor_copy(
                                        ot[:, nb * 512: (nb + 1) * 512], po[:])
                                nc.sync.dma_start(out_d[tsl, :], ot[:])
                                return
                            # kind == "g"
                            r = g - 4 * qb
                            lo = 128 * r if r > 0 else 0
                            pt = pt_store.pop(idx)
                            if (qb, p) not in y_store:
                                ya = psy.tile([65, 512], f32, tag="y",
                                              name=f"ya_{qb}_{p}")
                                yb = psy.tile([65, 512], f32, tag="y",
                                              name=f"yb_{qb}_{p}")
                                y_store[(qb, p)] = (ya, yb)
                            ya, yb = y_store[(qb, p)]
                            if do_av:
                                for hf, yy in ((0, ya), (1, yb)):
                                    nc.tensor.matmul(
                                        yy[:, lo:512],
                                        v_sb[:, g, 2 * p + hf, :],
                                        pt[:, hf, lo:512],
                                        start=(g == 0), stop=(g == ng - 1),
                                        skip_group_check=True,
                                    )
                            if g != ng - 1:
                                return
                            if not do_av:
                                nc.vector.memset(ya[:], 1.0)
                                nc.vector.memset(yb[:], 1.0)
                            # normalize: yt = y[0:64] * (1 / rowsum)
                            qsl = slice(qb * 512, (qb + 1) * 512)
                            for hi, yy in ((0, ya), (1, yb)):
                                rec = recp.tile([1, 512], f32, tag="rec",
                                                name=f"rec_{qb}_{p}_{hi}")
                                if norm == "gps":
                                    nc.vector.reciprocal_approx_fast(
                                        rec[0:1, :], yy[64:65, :])
                                else:  # "sbuf": stage rowsum in SBUF first
                                    rs = recp.tile([1, 512], f32, tag="rs",
                                                   name=f"rs_{qb}_{p}_{hi}")
                                    nc.vector.tensor_copy(
                                        rs[0:1, :], yy[64:65, :])
                                    nc.vector.reciprocal_approx_fast(
                                        rec[0:1, :], rs[0:1, :])
                                bc = bcp.tile([64, 512], f32, tag="bc",
                                              name=f"bc_{qb}_{p}_{hi}")
                                nc.gpsimd.partition_broadcast(
                                    bc[:], rec[0:1, :], channels=64)
                                nc.vector.tensor_mul(
                                    yt_sb[p][hi * 64: (hi + 1) * 64, qsl],
                                    yy[0:64, :], bc[:],
                                )
                            del y_store[(qb, p)]

                        n = len(tasks)
                        for j in range(min(la, n)):
                            emit_front(j)
                        for i in range(n):
                            if i + la < n:
                                emit_front(i + la)
                            emit_back(i)

                wqk_cm.__exit__(None, None, None)
                xtp_cm.__exit__(None, None, None)
                vp_cm.__exit__(None, None, None)

            if rep == 1:
                body()
            else:
                with tc.For_i(0, rep, 1):
                    body()

    nc.compile()
    return nc


@functools.lru_cache(maxsize=None)
def _get_nc(rep=1, la=5, sbufs=2, ybufs=2, pbufs=2, ptbufs=8, ablate="full",
            norm="sbuf", fp8a=False, xbufs=1, vbufs=1):
    return _build(rep, la, sbufs, ybufs, pbufs, ptbufs, ablate, norm, fp8a,
                  xbufs, vbufs)


FP8A = False  # must match the _build/_get_runner default


def make_in_maps(x, w_qkv, w_proj):
    import ml_dtypes
    bf16 = ml_dtypes.bfloat16
    j = np.arange(128)[None, :]
    i = np.arange(128)[:, None]
    tri = (j >= i).astype(bf16)

    in_maps = []
    for core in range(N_CORES):
        b, hg = divmod(core, HG)
        sl = slice(hg * 512, (hg + 1) * 512)
        xtb = np.ascontiguousarray(x[b].T)
        wqt = np.ascontiguousarray(w_qkv[sl].T)
        wkt = np.ascontiguousarray(w_qkv[C:2 * C][sl].T)
        m = {
            "xt": xtb.astype(bf16),
            "wq": wqt.astype(bf16),
            "wk": wkt.astype(bf16),
            "wv": np.ascontiguousarray(w_qkv[2 * C:3 * C][sl].T).astype(bf16),
            "wp": np.ascontiguousarray(w_proj[:, sl].T).astype(bf16),
            "tri": tri,
        }
        if FP8A:
            f8 = ml_dtypes.float8_e4m3

            def dr8(wt):
                # [C,512] -> DoubleRow-interleaved [128, kp*j*m] fp8, x32
                return np.ascontiguousarray(
                    (wt * 32.0).reshape(4, 2, 128, 512).transpose(2, 0, 1, 3)
                    .reshape(128, 4096)).astype(f8)

            m["xt8"] = xtb.astype(f8)
            m["wq8"] = dr8(wqt)
            m["wk8"] = dr8(wkt)
        in_maps.append(m)
    return in_maps


def combine(results):
    out = np.empty((B, T, C), dtype=np.float32)
    for b in range(B):
        out[b] = results[2 * b]["out"] + results[2 * b + 1]["out"]
    return out


# ---------------------------------------------------------------------------
# PJRT runner (device-resident inputs, reusable jitted executable)
# ---------------------------------------------------------------------------

class _Runner:
    def __init__(self, nc, n_cores=N_CORES):
        import jax
        import concourse.mybir as mybir
        from concourse import bass2jax
        from jax.sharding import Mesh, PartitionSpec, NamedSharding
        from jax.experimental.shard_map import shard_map

        self.jax = jax
        bass2jax.install_neuronx_cc_hook()
        partition_name = (
            nc.partition_id_tensor.name if nc.partition_id_tensor else None
        )
        in_names, out_names, out_avals, zero_outs = [], [], [], []
        for alloc in nc.m.functions[0].allocations:
            if not isinstance(alloc, mybir.MemoryLocationSet):
                continue
            name = alloc.memorylocations[0].name
            if alloc.kind == "ExternalInput":
                if name != partition_name:
                    in_names.append(name)
            elif alloc.kind == "ExternalOutput":
                out_names.append(name)
                shape = tuple(alloc.tensor_shape)
                dtype = mybir.dt.np(alloc.dtype)
                out_avals.append(jax.core.ShapedArray(shape, dtype))
                zero_outs.append(np.zeros(shape, dtype))
        self.in_names, self.out_names = in_names, out_names
        self.out_avals, self.zero_outs = out_avals, zero_outs
        self.n_cores = n_cores
        all_names = in_names + out_names
        if partition_name is not None:
            all_names = all_names + [partition_name]

        def _bdy(*args):
            operands = list(args)
            if partition_name is not None:
                operands.append(bass2jax.partition_id_tensor())
            outs = bass2jax._bass_exec_p.bind(
                *operands,
                out_avals=tuple(out_avals),
                in_names=tuple(all_names),
                out_names=tuple(out_names),
                lowering_input_output_aliases=(),
                sim_require_finite=True,
                sim_require_nnan=True,
                nc=nc,
            )
            return tuple(outs)

        devices = jax.devices()[:n_cores]
        mesh = Mesh(np.asarray(devices), ("core",))
        n_args = len(in_names) + len(out_names)
        self.fn = jax.jit(
            shard_map(
                _bdy, mesh=mesh,
                in_specs=(PartitionSpec("core"),) * n_args,
                out_specs=(PartitionSpec("core"),) * len(out_names),
                check_rep=False,
            ),
            keep_unused=True,
        )
        self.sharding = NamedSharding(mesh, PartitionSpec("core"))

    def put_inputs(self, in_maps):
        concat = [
            np.concatenate([np.asarray(m[name]) for m in in_maps], axis=0)
            for name in self.in_names
        ]
        concat += [
            np.zeros((self.n_cores * z.shape[0], *z.shape[1:]), z.dtype)
            for z in self.zero_outs
        ]
        self.args = [self.jax.device_put(a, self.sharding) for a in concat]
        self.jax.block_until_ready(self.args)

    def run(self):
        outs = self.fn(*self.args)
        self.jax.block_until_ready(outs)
        return [
            {
                name: np.asarray(outs[i]).reshape(
                    self.n_cores, *self.out_avals[i].shape)[c]
                for i, name in enumerate(self.out_names)
            }
            for c in range(self.n_cores)
        ]

    def time_ns(self, iters=20, warmup=2):
        import time
        for _ in range(warmup):
            self.jax.block_until_ready(self.fn(*self.args))
        t0 = time.perf_counter()
        outs = None
        for _ in range(iters):
            outs = self.fn(*self.args)
        self.jax.block_until_ready(outs)
        t1 = time.perf_counter()
        return (t1 - t0) / iters * 1e9



@functools.lru_cache(maxsize=None)
def _get_runner(rep=1, la=5, sbufs=2, ybufs=2, pbufs=2, ptbufs=8, ablate="full",
                norm="sbuf", fp8a=False, xbufs=1, vbufs=1):
    return _Runner(_get_nc(rep, la, sbufs, ybufs, pbufs, ptbufs, ablate, norm,
                           fp8a, xbufs, vbufs))


def kernel(x, w_qkv, w_proj):
    x = np.asarray(x, dtype=np.float32)
    w_qkv = np.asarray(w_qkv, dtype=np.float32)
    w_proj = np.asarray(w_proj, dtype=np.float32)
    runner = _get_runner()
    runner.put_inputs(make_in_maps(x, w_qkv, w_proj))
    return combine(runner.run())
